# revision 1
# baseline (speedup 1.0000x reference)
"""DeepseekV2 MLA attention on 8 Trainium2 NeuronCores (Bass/Tile), v4.

Token-sharded front end: each core computes q_a/latent + RMS-norm + RoPE and
the q_b/kv_b projections for ALL heads on its 256-token shard, then a single
AllToAll redistributes to head-sharded layout (2 heads/core, all tokens) for
attention and the row-parallel output projection. Host sums partials.
"""

import numpy as np

import concourse.bass as bass
import concourse.bacc as bacc
import concourse.mybir as mybir
import concourse.tile as tile
from concourse import bass_utils

T = 2048
HID = 2048
H = 16
DN = 128
DR = 64
DV = 128
DQK = DN + DR
QLR = 1536
KVLR = 512
THETA = 10000.0
EPS = 1e-6
SCALE = DQK ** -0.5

NCORES = 8
HPC = H // NCORES
LATR = KVLR + DR

F32 = mybir.dt.float32
F32R = mybir.dt.float32r

KT = HID // 128
QMT = QLR // 128
KVMT = KVLR // 128
NB = T // 512
TBT = T // 128
TSH = T // NCORES            # 256 tokens per shard

# per-dest chunk layouts for the two AllToAlls (rows):
# kv: kn h0 (128) | kn h1 (128) | v h0 (128) | v h1 (128) | kpe (64) = 576
# q:  qn h0 (128) | qn h1 (128) | qpe h0 (64) | qpe h1 (64)        = 384
KCH = 576
QCH = 384
OFF_KN = 0
OFF_V = 256
OFF_KPE = 512
OFF_QN = 0
OFF_QPE = 256


def build_bass():
    nc = bacc.Bacc(
        "TRN2",
        target_bir_lowering=False,
        debug=False,
        enable_asserts=False,
        num_devices=NCORES,
    )

    hs_sh = nc.dram_tensor("hs_sh", [HID, TSH], F32R, kind="ExternalInput").ap()
    wqa = nc.dram_tensor("wqa", [QMT * 128, KT * 128], F32R, kind="ExternalInput").ap()
    wkva = nc.dram_tensor("wkva", [KVMT * 128, KT * 128], F32R, kind="ExternalInput").ap()
    wkpe = nc.dram_tensor("wkpe", [128, KT * DR], F32R, kind="ExternalInput").ap()
    wqb = nc.dram_tensor("wqb", [NCORES * 128, QMT * HPC * DQK], F32R, kind="ExternalInput").ap()
    wkvbk = nc.dram_tensor("wkvbk", [NCORES * 128, KVMT * HPC * DN], F32R, kind="ExternalInput").ap()
    wkvbv = nc.dram_tensor("wkvbv", [NCORES * 128, KVMT * HPC * DV], F32R, kind="ExternalInput").ap()
    wo = nc.dram_tensor("wo", [HPC * DV, HID], F32R, kind="ExternalInput").ap()
    cosf_sh = nc.dram_tensor("cosf_sh", [DR, TSH], F32R, kind="ExternalInput").ap()
    sinf_sh = nc.dram_tensor("sinf_sh", [DR, TSH], F32R, kind="ExternalInput").ap()
    perm64 = nc.dram_tensor("perm64", [DR, DR], F32R, kind="ExternalInput").ap()
    ident = nc.dram_tensor("ident", [128, 128], F32R, kind="ExternalInput").ap()
    maskd = nc.dram_tensor("maskd", [128, 4 * 512], F32R, kind="ExternalInput").ap()
    ones = nc.dram_tensor("ones", [128, 128], F32R, kind="ExternalInput").ap()
    out = nc.dram_tensor("out", [T, HID], F32, kind="ExternalOutput").ap()

    with tile.TileContext(nc) as tc:
        _kernel_body(nc, tc, hs_sh, wqa, wkva, wkpe, wqb, wkvbk, wkvbv, wo,
                     cosf_sh, sinf_sh, perm64, ident, maskd, ones, out)

    nc.compile()
    return nc


def _kernel_body(nc, tc, hs_sh, wqa, wkva, wkpe, wqb, wkvbk, wkvbv, wo,
                 cosf_sh, sinf_sh, perm64, ident, maskd, ones, out):
    from contextlib import ExitStack

    ctx = ExitStack()
    with ctx:
        dram = ctx.enter_context(tc.tile_pool(name="dram", bufs=1, space="DRAM"))
        contrib_kv = dram.tile([NCORES * KCH, TSH], F32R)
        contrib_q = dram.tile([NCORES * QCH, TSH], F32R)
        a2a_kv = dram.tile([NCORES * KCH, TSH], F32R)
        a2a_q = dram.tile([NCORES * QCH, TSH], F32R)

        persist = ctx.enter_context(tc.tile_pool(name="persist", bufs=1))
        ones128 = persist.tile([128, 128], F32R, tag="ones128")
        nc.sync.dma_start(out=ones128, in_=ones)
        ones_col = ones128[:, 0:1]
        ones_row = ones128[0:1, :]
        perm_t = persist.tile([DR, DR], F32R, tag="perm0")
        nc.sync.dma_start(out=perm_t, in_=perm64)
        ident_t = persist.tile([128, 128], F32R, tag="ident")
        nc.sync.dma_start(out=ident_t, in_=ident)
        cosf_t = persist.tile([DR, TSH], F32R, tag="cosfsh")
        nc.sync.dma_start(out=cosf_t, in_=cosf_sh)
        sinf_t = persist.tile([DR, TSH], F32R, tag="sinfsh")
        nc.sync.dma_start(out=sinf_t, in_=sinf_sh)
        pmid = ctx.enter_context(tc.tile_pool(name="pmid", bufs=1))

        # ---- Phase A: shard q_a / latent, norm, local rope of k_pe ----------
        with tc.tile_pool(name="pa", bufs=1) as pa, \
             tc.tile_pool(name="psa", bufs=1, space="PSUM") as psa:
            hst = []
            for k in range(KT):
                h = pa.tile([128, TSH], F32R, tag=f"hs{k}")
                nc.sync.dma_start(out=h, in_=hs_sh[k * 128:(k + 1) * 128, :])
                hst.append(h)

            def a_mtile(w_src, mrows, z_tile, z_start, z_stop, stg_tag):
                wstrip = pa.tile([128, KT, mrows], F32R, tag="wstrip", bufs=2)
                nc.scalar.dma_start(
                    out=wstrip,
                    in_=w_src.rearrange("p (kc m) -> p kc m", kc=KT),
                )
                pq = psa.tile([mrows, TSH], F32, tag="pq", bufs=3)
                for k in range(KT):
                    nc.tensor.matmul(
                        pq, lhsT=wstrip[:, k, :], rhs=hst[k],
                        start=(k == 0), stop=(k == KT - 1))
                stage = pa.tile([mrows, TSH], F32R, tag=stg_tag, name=stg_tag)
                nc.vector.tensor_copy(stage, pq)
                if z_tile is not None:
                    sq = pa.tile([mrows, TSH], F32R, tag="sq", bufs=2)
                    nc.scalar.square(sq, stage)
                    nc.tensor.matmul(z_tile, lhsT=ones_col[0:mrows, :], rhs=sq,
                                     start=z_start, stop=z_stop)
                return stage

            def rsqrt_bc(z_psum, n, tag):
                tmp = pa.tile([1, TSH], F32, tag="rsq_tmp", bufs=2)
                nc.scalar.activation(tmp, z_psum,
                                     mybir.ActivationFunctionType.Copy,
                                     bias=EPS, scale=1.0 / n)
                nc.vector.reciprocal(tmp, tmp)
                srow = pa.tile([1, TSH], F32R, tag=tag + "r", name=tag + "r")
                nc.scalar.activation(srow, tmp,
                                     mybir.ActivationFunctionType.Sqrt)
                b_ps = psa.tile([128, TSH], F32, tag="bc", bufs=1)
                nc.tensor.matmul(b_ps, lhsT=ones_row, rhs=srow,
                                 start=True, stop=True)
                bc = pmid.tile([128, TSH], F32R, tag=tag, name=tag)
                nc.scalar.copy(bc, b_ps)
                return bc

            def rope_local(dst, raw, pool_ps):
                sw_ps = pool_ps.tile([DR, TSH], F32, tag="bc", bufs=1)
                nc.tensor.matmul(sw_ps, lhsT=perm_t, rhs=raw,
                                 start=True, stop=True)
                rt1 = pmid.tile([DR, TSH], F32R, tag="rt1", bufs=2)
                nc.vector.tensor_tensor(rt1, raw, cosf_t,
                                        op=mybir.AluOpType.mult)
                rt2 = pmid.tile([DR, TSH], F32R, tag="rt2", bufs=2)
                nc.vector.tensor_tensor(rt2, sw_ps, sinf_t,
                                        op=mybir.AluOpType.mult)
                nc.vector.tensor_tensor(dst, rt1, rt2, op=mybir.AluOpType.add)

            # kv side first
            zkv = psa.tile([1, TSH], F32, tag="z")
            kv_stages = []
            for m in range(KVMT):
                kv_stages.append(a_mtile(wkva[m * 128:(m + 1) * 128, :], 128,
                                         zkv, m == 0, m == KVMT - 1, f"stkv{m}"))
            kpe_stage = a_mtile(wkpe, DR, None, False, False, "stkpe")
            skv_bc = rsqrt_bc(zkv, KVLR, "skvbc")
            kvan = []
            for m in range(KVMT):
                kk = pmid.tile([128, TSH], F32R, tag=f"kvan{m}", name=f"kvan{m}")
                nc.vector.tensor_tensor(kk, kv_stages[m], skv_bc,
                                        op=mybir.AluOpType.mult)
                kvan.append(kk)
            kpel = pmid.tile([DR, TSH], F32R, tag="kpel")
            rope_local(kpel, kpe_stage, psa)
            # replicate roped k_pe into every dest chunk
            for d in range(NCORES):
                nc.sync.dma_start(
                    out=contrib_kv[d * KCH + OFF_KPE:d * KCH + OFF_KPE + DR, :],
                    in_=kpel)

            # q side
            zq = psa.tile([1, TSH], F32, tag="z")
            q_stages = []
            for m in range(QMT):
                q_stages.append(a_mtile(wqa[m * 128:(m + 1) * 128, :], 128,
                                        zq, m == 0, m == QMT - 1, f"stq{m}"))
            sq_bc = rsqrt_bc(zq, QLR, "sqbc")
            qan = []
            for m in range(QMT):
                qq = pmid.tile([128, TSH], F32R, tag=f"qan{m}", name=f"qan{m}")
                nc.vector.tensor_tensor(qq, q_stages[m], sq_bc,
                                        op=mybir.AluOpType.mult)
                qan.append(qq)

        # ---- kv_b projections for all dests + early kv exchange ----
        with tc.tile_pool(name="pw1", bufs=1) as pw1, \
             tc.tile_pool(name="psw1", bufs=1, space="PSUM") as psw1:
            for d in range(NCORES):
                wk = pw1.tile([128, KVMT, HPC * DN], F32R, tag="wk", bufs=2)
                nc.sync.dma_start(
                    out=wk,
                    in_=wkvbk[d * 128:(d + 1) * 128, :].rearrange(
                            "p (kc m) -> p kc m", kc=KVMT))
                wv = pw1.tile([128, KVMT, HPC * DV], F32R, tag="wv", bufs=2)
                nc.sync.dma_start(
                    out=wv,
                    in_=wkvbv[d * 128:(d + 1) * 128, :].rearrange(
                            "p (kc m) -> p kc m", kc=KVMT))
                for h in range(HPC):
                    acck = psw1.tile([128, TSH], F32, tag="acck", bufs=2,
                                     name="acck")
                    accv = psw1.tile([128, TSH], F32, tag="accv", bufs=2,
                                     name="accv")
                    for k in range(KVMT):
                        nc.tensor.matmul(
                            acck, lhsT=wk[:, k, h * DN:(h + 1) * DN],
                            rhs=kvan[k],
                            start=(k == 0), stop=(k == KVMT - 1))
                        nc.tensor.matmul(
                            accv, lhsT=wv[:, k, h * DV:(h + 1) * DV],
                            rhs=kvan[k],
                            start=(k == 0), stop=(k == KVMT - 1))
                    knt = pw1.tile([128, TSH], F32R, tag="knt", bufs=3)
                    nc.vector.tensor_copy(knt, acck)
                    nc.scalar.dma_start(
                        out=contrib_kv[d * KCH + OFF_KN + h * DN:
                                       d * KCH + OFF_KN + (h + 1) * DN, :],
                        in_=knt)
                    vtt = pw1.tile([128, TSH], F32R, tag="vtt", bufs=3)
                    nc.vector.tensor_copy(vtt, accv)
                    nc.scalar.dma_start(
                        out=contrib_kv[d * KCH + OFF_V + h * DV:
                                       d * KCH + OFF_V + (h + 1) * DV, :],
                        in_=vtt)
        nc.gpsimd.collective_compute(
            "AllToAll", mybir.AluOpType.bypass,
            replica_groups=[list(range(NCORES))],
            ins=[contrib_kv], outs=[a2a_kv])

        # ---- q_b projections for all dests + q exchange ------------------
        with tc.tile_pool(name="pw", bufs=1) as pw, \
             tc.tile_pool(name="psw", bufs=1, space="PSUM") as psw:
            for d in range(NCORES):
                wq = pw.tile([128, QMT, HPC * DQK], F32R, tag="wq", bufs=2)
                nc.sync.dma_start(
                    out=wq,
                    in_=wqb[d * 128:(d + 1) * 128, :].rearrange(
                        "p (kc m) -> p kc m", kc=QMT))
                col_of = (0, DN, 2 * DN, 2 * DN + DR)
                rows_of = (DN, DN, DR, DR)
                accq = []
                for mt in range(4):
                    a = psw.tile([rows_of[mt], TSH], F32, tag="acc",
                                 bufs=4, name=f"accq{mt}")
                    accq.append(a)
                for k in range(QMT):
                    for mt in range(4):
                        nc.tensor.matmul(
                            accq[mt],
                            lhsT=wq[:, k, col_of[mt]:col_of[mt] + rows_of[mt]],
                            rhs=qan[k],
                            start=(k == 0), stop=(k == QMT - 1))
                for h in range(HPC):
                    qnt = pw.tile([128, TSH], F32R, tag="qnt", bufs=3)
                    nc.vector.tensor_copy(qnt, accq[h])
                    nc.scalar.dma_start(
                        out=contrib_q[d * QCH + OFF_QN + h * DN:
                                      d * QCH + OFF_QN + (h + 1) * DN, :],
                        in_=qnt)
                    qraw = pw.tile([DR, TSH], F32R, tag="qraw", bufs=2)
                    nc.vector.tensor_copy(qraw, accq[2 + h])
                    qper = pw.tile([DR, TSH], F32R, tag="qper", bufs=3)
                    rope_local(qper, qraw, psw)
                    nc.scalar.dma_start(
                        out=contrib_q[d * QCH + OFF_QPE + h * DR:
                                      d * QCH + OFF_QPE + (h + 1) * DR, :],
                        in_=qper)
            nc.gpsimd.collective_compute(
                "AllToAll", mybir.AluOpType.bypass,
                replica_groups=[list(range(NCORES))],
                ins=[contrib_q], outs=[a2a_q])

        # ---- Phase B: load head-sharded tiles, transpose v ------------------
        qn = [[None] * NB for _ in range(HPC)]
        qpe = [[None] * NB for _ in range(HPC)]
        kn = [[None] * NB for _ in range(HPC)]
        kpe = [None] * NB
        vt = [None] * TBT

        bcp = ctx.enter_context(tc.tile_pool(name="bcp", bufs=1))

        with tc.tile_pool(name="pb", bufs=1) as pb, \
             tc.tile_pool(name="psb", bufs=1, space="PSUM") as psb:
            for j in range(NB):
                srcs = (2 * j, 2 * j + 1)
                for h in range(HPC):
                    qn[h][j] = bcp.tile([128, 512], F32R, tag=f"qn{h}_{j}",
                                        name=f"qn{h}_{j}")
                    qpe[h][j] = bcp.tile([DR, 512], F32R, tag=f"qpe{h}_{j}",
                                         name=f"qpe{h}_{j}")
                    kn[h][j] = bcp.tile([128, 512], F32R, tag=f"kn{h}_{j}",
                                        name=f"kn{h}_{j}")
                    for half, s in enumerate(srcs):
                        hsl = slice(half * TSH, (half + 1) * TSH)
                        nc.sync.dma_start(
                            out=kn[h][j][:, hsl],
                            in_=a2a_kv[s * KCH + OFF_KN + h * DN:
                                       s * KCH + OFF_KN + (h + 1) * DN, :])
                        nc.sync.dma_start(
                            out=qn[h][j][:, hsl],
                            in_=a2a_q[s * QCH + OFF_QN + h * DN:
                                      s * QCH + OFF_QN + (h + 1) * DN, :])
                        nc.scalar.dma_start(
                            out=qpe[h][j][:, hsl],
                            in_=a2a_q[s * QCH + OFF_QPE + h * DR:
                                      s * QCH + OFF_QPE + (h + 1) * DR, :])
                kpe[j] = bcp.tile([DR, 512], F32R, tag=f"kpe_{j}",
                                  name=f"kpe_{j}")
                for half, s in enumerate(srcs):
                    nc.scalar.dma_start(
                        out=kpe[j][:, half * TSH:(half + 1) * TSH],
                        in_=a2a_kv[s * KCH + OFF_KPE:s * KCH + OFF_KPE + DR, :])
                # v: feature-major per source -> transpose to token-major
                for half, s in enumerate(srcs):
                    vfs = []
                    for h in range(HPC):
                        vf = pb.tile([DV, TSH], F32R, tag="vf", bufs=4,
                                     name=f"vf{h}")
                        nc.sync.dma_start(
                            out=vf, in_=a2a_kv[s * KCH + OFF_V + h * DV:
                                               s * KCH + OFF_V + (h + 1) * DV, :])
                        vfs.append(vf)
                    for tt in range(2):
                        tb = j * 4 + half * 2 + tt
                        vt[tb] = bcp.tile([128, HPC * DV], F32R,
                                          tag=f"v_{tb}", name=f"v_{tb}")
                        for h in range(HPC):
                            tr = psb.tile([128, 128], F32R, tag="tr", bufs=4)
                            nc.tensor.transpose(
                                tr,
                                vfs[h][:, tt * 128:(tt + 1) * 128],
                                ident_t)
                            nc.vector.tensor_copy(
                                vt[tb][:, h * DV:(h + 1) * DV], tr)

        # ---- Attention + output projection ---------------------------------
        with tc.tile_pool(name="pc", bufs=1) as pc, \
             tc.tile_pool(name="psc", bufs=1, space="PSUM") as psc:
            maskd_t = pc.tile([128, 4 * 512], F32R, tag="maskd")
            nc.sync.dma_start(out=maskd_t, in_=maskd)
            wo_t = []
            for h in range(HPC):
                w = pc.tile([128, HID], F32R, tag=f"wo{h}")
                nc.sync.dma_start(out=w, in_=wo[h * DV:(h + 1) * DV, :])
                wo_t.append(w)

            attn_n = [[None] * NB for _ in range(HPC)]
            for qj in range(NB):
                nki = 4 * qj + 4
                for h in range(HPC):
                    attn_ps = psc.tile([128, 512], F32, tag="attn", bufs=2)
                    z_ps = psc.tile([1, 512], F32, tag="zr", bufs=1)
                    for ki in range(nki):
                        jb, sub = ki // 4, ki % 4
                        ksl = slice(sub * 128, (sub + 1) * 128)
                        s_ps = psc.tile([128, 512], F32, tag="s", bufs=3)
                        nc.tensor.matmul(s_ps, lhsT=kn[h][jb][:, ksl],
                                         rhs=qn[h][qj],
                                         start=True, stop=False)
                        nc.tensor.matmul(s_ps, lhsT=kpe[jb][:, ksl],
                                         rhs=qpe[h][qj],
                                         start=False, stop=True)
                        e = pc.tile([128, 512], F32R, tag="e", bufs=4)
                        nc.scalar.activation(e, s_ps,
                                             mybir.ActivationFunctionType.Exp)
                        if ki >= 4 * qj:
                            sub_d = ki - 4 * qj
                            nc.vector.tensor_tensor(
                                e, e, maskd_t[:, sub_d * 512:(sub_d + 1) * 512],
                                op=mybir.AluOpType.mult)
                        nc.tensor.matmul(z_ps, lhsT=ones_col, rhs=e,
                                         start=(ki == 0), stop=(ki == nki - 1))
                        nc.tensor.matmul(attn_ps,
                                         lhsT=vt[ki][:, h * DV:(h + 1) * DV],
                                         rhs=e,
                                         start=(ki == 0), stop=(ki == nki - 1))
                    rz = pc.tile([1, 512], F32R, tag="rz", bufs=2)
                    with nc.allow_low_precision(reason="fp32r softmax denom"):
                        nc.vector.reciprocal(rz, z_ps)
                    bc_ps = psc.tile([128, 512], F32, tag="s", bufs=3)
                    nc.tensor.matmul(bc_ps, lhsT=ones_row, rhs=rz,
                                     start=True, stop=True)
                    bc_sb = pc.tile([128, 512], F32R, tag="bcs", bufs=2)
                    nc.scalar.copy(bc_sb, bc_ps)
                    attn_n[h][qj] = bcp.tile([128, 512], F32R,
                                             tag=f"attn{h}_{qj}",
                                             name=f"attn{h}_{qj}")
                    nc.vector.tensor_tensor(attn_n[h][qj], attn_ps, bc_sb,
                                            op=mybir.AluOpType.mult)

                for tt in range(4):
                    tb = qj * 4 + tt
                    tsl = slice(tt * 128, (tt + 1) * 128)
                    o_row = pc.tile([128, HID], F32, tag="orow", bufs=2)
                    for hb in range(NB):
                        o_ps = psc.tile([128, 512], F32, tag="o", bufs=2)
                        for h in range(HPC):
                            nc.tensor.matmul(
                                o_ps,
                                lhsT=attn_n[h][qj][:, tsl],
                                rhs=wo_t[h][:, hb * 512:(hb + 1) * 512],
                                start=(h == 0),
                                stop=(h == HPC - 1),
                            )
                        nc.vector.tensor_copy(
                            o_row[:, hb * 512:(hb + 1) * 512], o_ps)
                    nc.scalar.dma_start(
                        out=out[tb * 128:(tb + 1) * 128, :], in_=o_row)


_NC_CACHE = {}


def _get_nc():
    if "nc" not in _NC_CACHE:
        _NC_CACHE["nc"] = build_bass()
    return _NC_CACHE["nc"]


def make_in_maps(positions, hidden_states, w_q_a, q_a_ln_w, w_q_b, w_kv_a,
                 kv_a_ln_w, w_kv_b, w_o):
    positions = np.asarray(positions)
    hidden_states = np.asarray(hidden_states, dtype=np.float32)
    w_q_a = np.asarray(w_q_a, dtype=np.float32)
    q_a_ln_w = np.asarray(q_a_ln_w, dtype=np.float32)
    w_q_b = np.asarray(w_q_b, dtype=np.float32)
    w_kv_a = np.asarray(w_kv_a, dtype=np.float32)
    kv_a_ln_w = np.asarray(kv_a_ln_w, dtype=np.float32)
    w_kv_b = np.asarray(w_kv_b, dtype=np.float32)
    w_o = np.asarray(w_o, dtype=np.float32)

    hs_t = np.ascontiguousarray(hidden_states.T)

    order = np.concatenate([np.arange(0, DR, 2), np.arange(1, DR, 2)])

    wkva_p = w_kv_a.copy()
    wkva_p[:, KVLR:] = w_kv_a[:, KVLR:][:, order]
    wkva_p = np.ascontiguousarray(wkva_p)

    inv_freq = 1.0 / (THETA ** (np.arange(0, DR, 2, dtype=np.float64) / DR))
    ang = positions.astype(np.float64)[:, None] * inv_freq[None, :]
    cosT = np.cos(ang).T.astype(np.float32)
    sinT = np.sin(ang).T.astype(np.float32)
    cosf = np.ascontiguousarray(np.concatenate([cosT, cosT], axis=0))
    sinf = np.ascontiguousarray(np.concatenate([-sinT, sinT], axis=0))

    perm = np.zeros((DR, DR), dtype=np.float32)
    for i in range(DR):
        perm[i, (i + DR // 2) % DR] = 1.0

    maskd = np.zeros((128, 4 * 512), dtype=np.float32)
    p = np.arange(128)[:, None]
    f = np.arange(512)[None, :]
    for sub in range(4):
        maskd[:, sub * 512:(sub + 1) * 512] = (p + 128 * sub <= f)
    maskd = np.ascontiguousarray(maskd)

    # all-heads b-weights, columns grouped per destination core
    wqb_all = np.concatenate([
        np.concatenate([
            w_q_b[:, h0 * DQK:h0 * DQK + DN],
            w_q_b[:, h1 * DQK:h1 * DQK + DN],
            w_q_b[:, h0 * DQK + DN:(h0 + 1) * DQK][:, order],
            w_q_b[:, h1 * DQK + DN:(h1 + 1) * DQK][:, order],
        ], axis=1)
        for h0, h1 in ((2 * d, 2 * d + 1) for d in range(NCORES))
    ], axis=1) * q_a_ln_w[:, None] * SCALE
    wkvbk_all = np.concatenate([
        w_kv_b[:, h * (DN + DV):h * (DN + DV) + DN] for h in range(H)
    ], axis=1) * kv_a_ln_w[:, None]
    wkvbv_all = np.concatenate([
        w_kv_b[:, h * (DN + DV) + DN:(h + 1) * (DN + DV)] for h in range(H)
    ], axis=1) * kv_a_ln_w[:, None]

    def pack(w, mrows):
        # [K, M] -> strip-major [nstrips*128, (K/128)*mrows]: each strip row-
        # contiguous so the device DMA is 128 fat descriptors
        Kd, Md = w.shape
        n = Md // mrows
        return np.ascontiguousarray(
            w.reshape(Kd // 128, 128, n, mrows).transpose(2, 1, 0, 3)
            .reshape(n * 128, (Kd // 128) * mrows).astype(np.float32))

    wqa_pk = pack(w_q_a, 128)
    wkva_pk = pack(wkva_p[:, :KVLR], 128)
    wkpe_pk = pack(wkva_p[:, KVLR:], DR)
    wqb_pk = pack(wqb_all.astype(np.float32), HPC * DQK)
    wkvbk_pk = pack(wkvbk_all.astype(np.float32), HPC * DN)
    wkvbv_pk = pack(wkvbv_all.astype(np.float32), HPC * DV)

    in_maps = []
    for c in range(NCORES):
        h0, h1 = HPC * c, HPC * c + 1
        wo_c = np.concatenate([
            w_o[h0 * DV:(h0 + 1) * DV, :],
            w_o[h1 * DV:(h1 + 1) * DV, :],
        ], axis=0)
        tsl = slice(c * TSH, (c + 1) * TSH)
        in_maps.append({
            "hs_sh": np.ascontiguousarray(hs_t[:, tsl]),
            "wqa": wqa_pk,
            "wkva": wkva_pk,
            "wkpe": wkpe_pk,
            "wqb": wqb_pk,
            "wkvbk": wkvbk_pk,
            "wkvbv": wkvbv_pk,
            "wo": np.ascontiguousarray(wo_c.astype(np.float32)),
            "cosf_sh": np.ascontiguousarray(cosf[:, tsl]),
            "sinf_sh": np.ascontiguousarray(sinf[:, tsl]),
            "perm64": perm,
            "ident": np.eye(128, dtype=np.float32),
            "maskd": maskd,
            "ones": np.ones((128, 128), dtype=np.float32),
        })
    return in_maps


def kernel(positions, hidden_states, w_q_a, q_a_ln_w, w_q_b, w_kv_a,
           kv_a_ln_w, w_kv_b, w_o):
    nc = _get_nc()
    in_maps = make_in_maps(positions, hidden_states, w_q_a, q_a_ln_w, w_q_b,
                           w_kv_a, kv_a_ln_w, w_kv_b, w_o)
    res = bass_utils.run_bass_kernel_spmd(nc, in_maps, core_ids=list(range(NCORES)))
    acc = np.zeros((T, HID), dtype=np.float32)
    for c in range(NCORES):
        acc += res.results[c]["out"]
    return acc



# revision 6
# speedup vs baseline: 1.6893x; 1.6893x over previous
"""DeepseekV2 MLA attention on 8 Trainium2 NeuronCores (Bass/Tile), v5.

All-bf16 datapath (fp32 PSUM).  Token-sharded front end computes the q/kv
latents on its 256-token shard; the 576-row kv latent (normalized kv_a +
roped k_pe) is AllGathered early so each core expands k_nope/v for its own
2 heads over all 2048 tokens while the q_b outputs (all heads, own shard)
are AllToAll'd.  Attention + row-parallel w_o as in v4; causal mask is
applied additively in PSUM via an identity matmul; host sums partials.
"""

import numpy as np
import ml_dtypes

import concourse.bass as bass
import concourse.bacc as bacc
import concourse.mybir as mybir
import concourse.tile as tile
from concourse import bass_utils

T = 2048
HID = 2048
H = 16
DN = 128
DR = 64
DV = 128
DQK = DN + DR
QLR = 1536
KVLR = 512
THETA = 10000.0
EPS = 1e-6
SCALE = DQK ** -0.5

NCORES = 8
HPC = H // NCORES
LATR = KVLR + DR          # 576 rows of exchanged kv latent

F32 = mybir.dt.float32
BF = mybir.dt.bfloat16
BF_NP = ml_dtypes.bfloat16

KT = HID // 128           # 16 contraction strips over hidden
QMT = QLR // 128          # 12
KVMT = KVLR // 128        # 4
NB = T // 512             # 4 query blocks
TBT = T // 128            # 16 token blocks
TSH = T // NCORES         # 256 tokens per shard

QCH = 3 * 128             # 384 rows per dest in the q exchange
MASKV = -60.0


def build_bass():
    nc = bacc.Bacc(
        "TRN2",
        target_bir_lowering=False,
        debug=False,
        enable_asserts=False,
        num_devices=NCORES,
    )

    hs_sh = nc.dram_tensor("hs_sh", [HID, TSH], BF, kind="ExternalInput").ap()
    wqa = nc.dram_tensor("wqa", [QMT * 128, KT * 128], BF, kind="ExternalInput").ap()
    wkva = nc.dram_tensor("wkva", [KVMT * 128, KT * 128], BF, kind="ExternalInput").ap()
    wkpe = nc.dram_tensor("wkpe", [128, KT * DR], BF, kind="ExternalInput").ap()
    wqb = nc.dram_tensor("wqb", [NCORES * 128, QMT * QCH], BF, kind="ExternalInput").ap()
    wkvb = nc.dram_tensor("wkvb", [128, KVMT * 4 * 128], BF, kind="ExternalInput").ap()
    wo = nc.dram_tensor("wo", [HPC * DV, HID], BF, kind="ExternalInput").ap()
    cosf2 = nc.dram_tensor("cosf2", [128, TSH], BF, kind="ExternalInput").ap()
    sinf2 = nc.dram_tensor("sinf2", [128, TSH], BF, kind="ExternalInput").ap()
    perm128 = nc.dram_tensor("perm128", [128, 128], BF, kind="ExternalInput").ap()
    ident = nc.dram_tensor("ident", [128, 128], BF, kind="ExternalInput").ap()
    maskd = nc.dram_tensor("maskd", [128, 4 * 512], BF, kind="ExternalInput").ap()
    ones = nc.dram_tensor("ones", [128, 128], BF, kind="ExternalInput").ap()
    out = nc.dram_tensor("out", [T, HID], BF, kind="ExternalOutput").ap()

    with tile.TileContext(nc) as tc:
        _kernel_body(nc, tc, hs_sh, wqa, wkva, wkpe, wqb, wkvb, wo,
                     cosf2, sinf2, perm128, ident, maskd, ones, out)

    nc.compile()
    return nc


def _kernel_body(nc, tc, hs_sh, wqa, wkva, wkpe, wqb, wkvb, wo,
                 cosf2, sinf2, perm128, ident, maskd, ones, out):
    from contextlib import ExitStack

    ctx = ExitStack()
    with ctx:
        dram = ctx.enter_context(tc.tile_pool(name="dram", bufs=1, space="DRAM"))
        contrib_kv = dram.tile([LATR, TSH], BF)
        a2a_kv = dram.tile([NCORES * LATR, TSH], BF)
        contrib_q = dram.tile([NCORES * QCH, TSH], BF)
        a2a_q = dram.tile([NCORES * QCH, TSH], BF)

        persist = ctx.enter_context(tc.tile_pool(name="persist", bufs=1))
        ident_t = persist.tile([128, 128], BF, tag="ident")
        nc.gpsimd.dma_start(out=ident_t, in_=ident)
        perm_t = persist.tile([128, 128], BF, tag="perm")
        nc.gpsimd.dma_start(out=perm_t, in_=perm128)
        cos_t = persist.tile([128, TSH], BF, tag="cos")
        nc.gpsimd.dma_start(out=cos_t, in_=cosf2)
        sin_t = persist.tile([128, TSH], BF, tag="sin")
        nc.gpsimd.dma_start(out=sin_t, in_=sinf2)
        ones_t = persist.tile([128, 128], BF, tag="ones")
        nc.gpsimd.dma_start(out=ones_t, in_=ones)
        maskd_t = persist.tile([128, 4 * 512], BF, tag="maskd")
        nc.gpsimd.dma_start(out=maskd_t, in_=maskd)
        wkvb_t = persist.tile([128, KVMT, 4 * 128], BF, tag="wkvb")
        nc.gpsimd.dma_start(
            out=wkvb_t, in_=wkvb.rearrange("p (s c) -> p s c", s=KVMT))
        wo_t = []
        for h in range(HPC):
            w = persist.tile([128, HID], BF, tag=f"wo{h}")
            nc.gpsimd.dma_start(out=w, in_=wo[h * DV:(h + 1) * DV, :])
            wo_t.append(w)
        ones_col = ones_t[:, 0:1]
        ones_row = ones_t[0:1, :]

        pmid = ctx.enter_context(tc.tile_pool(name="pmid", bufs=1))

        # ---- Phase A: latents on own shard --------------------------------
        with tc.tile_pool(name="pa", bufs=1) as pa, \
             tc.tile_pool(name="psa", bufs=1, space="PSUM") as psa:
            hst = []
            for k in range(KT):
                h = pa.tile([128, TSH], BF, tag=f"hs{k}")
                nc.sync.dma_start(out=h, in_=hs_sh[k * 128:(k + 1) * 128, :])
                hst.append(h)

            def rsqrt_bc(z_psum, n, tag):
                tmp = pa.tile([1, TSH], F32, tag="rsq_tmp", bufs=2)
                nc.scalar.activation(tmp, z_psum,
                                     mybir.ActivationFunctionType.Copy,
                                     bias=EPS, scale=1.0 / n)
                nc.vector.reciprocal(tmp, tmp)
                srow = pa.tile([1, TSH], BF, tag=tag + "r", name=tag + "r")
                nc.scalar.activation(srow, tmp,
                                     mybir.ActivationFunctionType.Sqrt)
                b_ps = psa.tile([128, TSH], F32, tag="bc", bufs=1)
                nc.tensor.matmul(b_ps, lhsT=ones_row, rhs=srow,
                                 start=True, stop=True)
                bc = pmid.tile([128, TSH], BF, tag=tag, name=tag)
                nc.scalar.copy(bc, b_ps)
                return bc

            # kv latent first (feeds the early AllGather)
            wkva_t = []
            for m in range(KVMT):
                wt = pa.tile([128, KT * 128], BF, tag="wkva", bufs=4,
                             name=f"wkva{m}")
                nc.sync.dma_start(out=wt, in_=wkva[m * 128:(m + 1) * 128, :])
                wkva_t.append(wt)
            wkpe_t = pa.tile([128, KT * DR], BF, tag="wkpe")
            nc.sync.dma_start(out=wkpe_t, in_=wkpe)

            zkv = psa.tile([1, TSH], F32, tag="zkv")
            kv_raw = []   # bf16 un-normalized latent strips
            for m in range(KVMT):
                pq = psa.tile([128, TSH], F32, tag="pq", bufs=3)
                for k in range(KT):
                    nc.tensor.matmul(pq, lhsT=wkva_t[m][:, k * 128:(k + 1) * 128],
                                     rhs=hst[k],
                                     start=(k == 0), stop=(k == KT - 1))
                st = pa.tile([128, TSH], BF, tag=f"kvr{m}", name=f"kvr{m}")
                nc.vector.tensor_copy(st, pq)
                kv_raw.append(st)
                sq = pa.tile([128, TSH], BF, tag="sq", bufs=2)
                nc.vector.tensor_tensor(sq, st, st, op=mybir.AluOpType.mult)
                nc.tensor.matmul(zkv, lhsT=ones_col, rhs=sq,
                                 start=(m == 0), stop=(m == KVMT - 1))
            # raw k_pe
            kpe_ps = psa.tile([DR, TSH], F32, tag="kpeps")
            for k in range(KT):
                nc.tensor.matmul(kpe_ps, lhsT=wkpe_t[:, k * DR:(k + 1) * DR],
                                 rhs=hst[k],
                                 start=(k == 0), stop=(k == KT - 1))
            kpe_raw = pa.tile([DR, TSH], BF, tag="kperaw")
            nc.vector.tensor_copy(kpe_raw, kpe_ps)

            skv_bc = rsqrt_bc(zkv, KVLR, "skvbc")
            # normalized latent staged contiguously for one contrib DMA
            kvstage = pa.tile([128, KVMT, TSH], BF, tag="kvstage")
            for m in range(KVMT):
                nc.vector.tensor_tensor(kvstage[:, m, :], kv_raw[m], skv_bc,
                                        op=mybir.AluOpType.mult)
            # rope k_pe (64 rows; use top half of perm/cos/sin)
            sw_ps = psa.tile([DR, TSH], F32, tag="swk")
            nc.tensor.matmul(sw_ps, lhsT=perm_t[0:DR, 0:DR], rhs=kpe_raw,
                             start=True, stop=True)
            rt1 = pa.tile([DR, TSH], BF, tag="rt1")
            nc.vector.tensor_tensor(rt1, kpe_raw, cos_t[0:DR, :],
                                    op=mybir.AluOpType.mult)
            rt2 = pa.tile([DR, TSH], BF, tag="rt2")
            nc.vector.tensor_tensor(rt2, sw_ps, sin_t[0:DR, :],
                                    op=mybir.AluOpType.mult)
            kpel = pa.tile([DR, TSH], BF, tag="kpel")
            nc.vector.tensor_tensor(kpel, rt1, rt2, op=mybir.AluOpType.add)

            nc.gpsimd.dma_start(
                out=contrib_kv[0:KVLR, :].rearrange("(g p) t -> p g t", p=128),
                in_=kvstage)
            nc.gpsimd.dma_start(out=contrib_kv[KVLR:LATR, :], in_=kpel)
            nc.gpsimd.collective_compute(
                "AllGather", mybir.AluOpType.bypass,
                replica_groups=[list(range(NCORES))],
                ins=[contrib_kv], outs=[a2a_kv])

            # q latent
            zq = psa.tile([1, TSH], F32, tag="zq")
            q_raw = []
            for m in range(QMT):
                wt = pa.tile([128, KT * 128], BF, tag="wqa", bufs=3)
                nc.sync.dma_start(out=wt, in_=wqa[m * 128:(m + 1) * 128, :])
                pq = psa.tile([128, TSH], F32, tag="pq", bufs=3)
                for k in range(KT):
                    nc.tensor.matmul(pq, lhsT=wt[:, k * 128:(k + 1) * 128],
                                     rhs=hst[k],
                                     start=(k == 0), stop=(k == KT - 1))
                st = pmid.tile([128, TSH], BF, tag=f"qr{m}", name=f"qr{m}")
                nc.vector.tensor_copy(st, pq)
                q_raw.append(st)
                sq = pa.tile([128, TSH], BF, tag="sq", bufs=2)
                nc.vector.tensor_tensor(sq, st, st, op=mybir.AluOpType.mult)
                nc.tensor.matmul(zq, lhsT=ones_col, rhs=sq,
                                 start=(m == 0), stop=(m == QMT - 1))
            sq_bc = rsqrt_bc(zq, QLR, "sqbc")
            qan = []
            for m in range(QMT):
                qq = pmid.tile([128, TSH], BF, tag=f"qan{m}", name=f"qan{m}")
                nc.vector.tensor_tensor(qq, q_raw[m], sq_bc,
                                        op=mybir.AluOpType.mult)
                qan.append(qq)

        # ---- q_b for all dests (3 x 128-row tiles per dest) + exchange ----
        with tc.tile_pool(name="pw", bufs=1) as pw, \
             tc.tile_pool(name="psw", bufs=1, space="PSUM") as psw:
            qstage = pw.tile([128, NCORES, 3, TSH], BF, tag="qstage")
            for d in range(NCORES):
                wq = pw.tile([128, QMT * QCH], BF, tag="wq", bufs=2)
                nc.sync.dma_start(out=wq, in_=wqb[d * 128:(d + 1) * 128, :])
                accq = []
                for mt in range(3):
                    a = psw.tile([128, TSH], F32, tag="acc", bufs=6,
                                 name=f"accq{mt}")
                    accq.append(a)
                for k in range(QMT):
                    for mt in range(3):
                        nc.tensor.matmul(
                            accq[mt],
                            lhsT=wq[:, k * QCH + mt * 128:k * QCH + (mt + 1) * 128],
                            rhs=qan[k],
                            start=(k == 0), stop=(k == QMT - 1))
                for hh in range(HPC):
                    nc.vector.tensor_copy(qstage[:, d, hh, :], accq[hh])
                # packed q_pe rope (two heads in one 128-row tile)
                qraw = pw.tile([128, TSH], BF, tag="qraw", bufs=2)
                nc.vector.tensor_copy(qraw, accq[2])
                sw = psw.tile([128, TSH], F32, tag="swq", bufs=2)
                nc.tensor.matmul(sw, lhsT=perm_t, rhs=qraw,
                                 start=True, stop=True)
                r1 = pw.tile([128, TSH], BF, tag="r1", bufs=2)
                nc.vector.tensor_tensor(r1, qraw, cos_t,
                                        op=mybir.AluOpType.mult)
                r2 = pw.tile([128, TSH], BF, tag="r2", bufs=2)
                nc.vector.tensor_tensor(r2, sw, sin_t,
                                        op=mybir.AluOpType.mult)
                nc.vector.tensor_tensor(qstage[:, d, 2, :], r1, r2,
                                        op=mybir.AluOpType.add)
            nc.sync.dma_start(
                out=contrib_q.rearrange("(d g p) t -> p d g t", d=NCORES, p=128),
                in_=qstage)
            nc.gpsimd.collective_compute(
                "AllToAll", mybir.AluOpType.bypass,
                replica_groups=[list(range(NCORES))],
                ins=[contrib_q], outs=[a2a_q])

        # ---- Phase B: expand k_nope / v for own heads over all tokens -----
        bcp = ctx.enter_context(tc.tile_pool(name="bcp", bufs=1))
        kvan = []      # latent strips, all tokens [128, 8, 256]
        for r in range(KVMT):
            kt_ = bcp.tile([128, NCORES, TSH], BF, tag=f"kvan{r}",
                           name=f"kvan{r}")
            nc.gpsimd.dma_start(
                out=kt_,
                in_=a2a_kv.rearrange("(s r) t -> r s t", s=NCORES)
                            [r * 128:(r + 1) * 128])
            kvan.append(kt_)
        kpe_all = bcp.tile([DR, NCORES, TSH], BF, tag="kpe")
        nc.gpsimd.dma_start(
            out=kpe_all,
            in_=a2a_kv.rearrange("(s r) t -> r s t", s=NCORES)[KVLR:LATR])

        def tok512(tile3, c):
            # 512-token chunk c of a [*, 8, 256] tile
            return tile3[:, 2 * c:2 * c + 2, :]

        def tok128(tile3, tb):
            half = (tb % 2) * 128
            return tile3[:, tb // 2, half:half + 128]

        kn = []        # per head [128, 8, 256] feature-major k_nope
        vt = [None] * TBT   # per 128-token block [128, HPC*DV] token-major v
        with tc.tile_pool(name="pb", bufs=1) as pb, \
             tc.tile_pool(name="psb", bufs=1, space="PSUM") as psb:
            for h in range(HPC):
                knh = bcp.tile([128, NCORES, TSH], BF, tag=f"kn{h}",
                               name=f"kn{h}")
                for c in range(4):
                    acck = psb.tile([128, 512], F32, tag="acck", bufs=2)
                    for s in range(KVMT):
                        nc.tensor.matmul(
                            acck, lhsT=wkvb_t[:, s, h * DN:(h + 1) * DN],
                            rhs=tok512(kvan[s], c),
                            start=(s == 0), stop=(s == KVMT - 1))
                    nc.vector.tensor_copy(tok512(knh, c), acck)
                kn.append(knh)
            for tb in range(TBT):
                accv = psb.tile([128, HPC * DV], F32, tag="accv", bufs=3)
                for s in range(KVMT):
                    nc.tensor.matmul(
                        accv, lhsT=tok128(kvan[s], tb),
                        rhs=wkvb_t[:, s, 2 * DN:2 * DN + HPC * DV],
                        start=(s == 0), stop=(s == KVMT - 1))
                vt[tb] = bcp.tile([128, HPC * DV], BF, tag=f"v{tb}",
                                  name=f"v{tb}")
                nc.vector.tensor_copy(vt[tb], accv)

        # q tiles for own heads, all tokens
        qn = []
        for h in range(HPC):
            qh = bcp.tile([128, NCORES, TSH], BF, tag=f"qn{h}", name=f"qn{h}")
            nc.sync.dma_start(
                out=qh,
                in_=a2a_q.rearrange("(s c) t -> c s t", s=NCORES)
                          [h * 128:(h + 1) * 128])
            qn.append(qh)
        qpe = []
        for h in range(HPC):
            qp = bcp.tile([DR, NCORES, TSH], BF, tag=f"qpe{h}", name=f"qpe{h}")
            nc.sync.dma_start(
                out=qp,
                in_=a2a_q.rearrange("(s c) t -> c s t", s=NCORES)
                          [2 * 128 + h * DR:2 * 128 + (h + 1) * DR])
            qpe.append(qp)

        # ---- Attention + output projection --------------------------------
        with tc.tile_pool(name="pc", bufs=1) as pc, \
             tc.tile_pool(name="psc", bufs=1, space="PSUM") as psc:
            attn_n = [[None] * NB for _ in range(HPC)]
            for qj in range(NB):
                nki = 4 * qj + 4
                for h in range(HPC):
                    attn_ps = psc.tile([128, 512], F32, tag="attn", bufs=2)
                    z_ps = psc.tile([1, 512], F32, tag="zr", bufs=1)
                    for ki in range(nki):
                        s_ps = psc.tile([128, 512], F32, tag="s", bufs=2)
                        diag = ki >= 4 * qj
                        nc.tensor.matmul(s_ps, lhsT=tok128(kn[h], ki),
                                         rhs=tok512(qn[h], qj),
                                         start=True, stop=False)
                        nc.tensor.matmul(s_ps, lhsT=tok128(kpe_all, ki),
                                         rhs=tok512(qpe[h], qj),
                                         start=False, stop=not diag)
                        if diag:
                            sub = ki - 4 * qj
                            nc.tensor.matmul(
                                s_ps, lhsT=ident_t,
                                rhs=maskd_t[:, sub * 512:(sub + 1) * 512],
                                start=False, stop=True)
                        e = pc.tile([128, 512], BF, tag="e", bufs=4)
                        nc.scalar.activation(e, s_ps,
                                             mybir.ActivationFunctionType.Exp)
                        nc.tensor.matmul(z_ps, lhsT=ones_col, rhs=e,
                                         start=(ki == 0), stop=(ki == nki - 1))
                        nc.tensor.matmul(attn_ps,
                                         lhsT=vt[ki][:, h * DV:(h + 1) * DV],
                                         rhs=e,
                                         start=(ki == 0), stop=(ki == nki - 1))
                    rz = pc.tile([1, 512], BF, tag="rz", bufs=2)
                    with nc.allow_low_precision(reason="bf16 softmax denom"):
                        nc.vector.reciprocal(rz, z_ps)
                    bc_ps = psc.tile([128, 512], F32, tag="bcs", bufs=1)
                    nc.tensor.matmul(bc_ps, lhsT=ones_row, rhs=rz,
                                     start=True, stop=True)
                    bc_sb = pc.tile([128, 512], BF, tag="bcsb", bufs=2)
                    nc.scalar.copy(bc_sb, bc_ps)
                    attn_n[h][qj] = bcp.tile([128, 512], BF,
                                             tag=f"attn{h}_{qj}",
                                             name=f"attn{h}_{qj}")
                    nc.vector.tensor_tensor(attn_n[h][qj], attn_ps, bc_sb,
                                            op=mybir.AluOpType.mult)

                for tt in range(4):
                    tb = qj * 4 + tt
                    tsl = slice(tt * 128, (tt + 1) * 128)
                    o_row = pc.tile([128, HID], BF, tag="orow", bufs=2)
                    for hb in range(NB):
                        o_ps = psc.tile([128, 512], F32, tag="o", bufs=2)
                        for h in range(HPC):
                            nc.tensor.matmul(
                                o_ps,
                                lhsT=attn_n[h][qj][:, tsl],
                                rhs=wo_t[h][:, hb * 512:(hb + 1) * 512],
                                start=(h == 0),
                                stop=(h == HPC - 1),
                            )
                        nc.vector.tensor_copy(
                            o_row[:, hb * 512:(hb + 1) * 512], o_ps)
                    eng = nc.sync if tb % 2 == 0 else nc.gpsimd
                    eng.dma_start(out=out[tb * 128:(tb + 1) * 128, :],
                                  in_=o_row)


_NC_CACHE = {}


def _get_nc():
    if "nc" not in _NC_CACHE:
        _NC_CACHE["nc"] = build_bass()
    return _NC_CACHE["nc"]


def make_in_maps(positions, hidden_states, w_q_a, q_a_ln_w, w_q_b, w_kv_a,
                 kv_a_ln_w, w_kv_b, w_o):
    positions = np.asarray(positions)
    hidden_states = np.asarray(hidden_states, dtype=np.float32)
    w_q_a = np.asarray(w_q_a, dtype=np.float32)
    q_a_ln_w = np.asarray(q_a_ln_w, dtype=np.float32)
    w_q_b = np.asarray(w_q_b, dtype=np.float32)
    w_kv_a = np.asarray(w_kv_a, dtype=np.float32)
    kv_a_ln_w = np.asarray(kv_a_ln_w, dtype=np.float32)
    w_kv_b = np.asarray(w_kv_b, dtype=np.float32)
    w_o = np.asarray(w_o, dtype=np.float32)

    hs_t = np.ascontiguousarray(hidden_states.T)

    order = np.concatenate([np.arange(0, DR, 2), np.arange(1, DR, 2)])

    wkva_p = w_kv_a.copy()
    wkva_p[:, KVLR:] = w_kv_a[:, KVLR:][:, order]

    inv_freq = 1.0 / (THETA ** (np.arange(0, DR, 2, dtype=np.float64) / DR))
    ang = positions.astype(np.float64)[:, None] * inv_freq[None, :]
    cosT = np.cos(ang).T.astype(np.float32)
    sinT = np.sin(ang).T.astype(np.float32)
    cosf = np.concatenate([cosT, cosT], axis=0)          # [64, T]
    sinf = np.concatenate([-sinT, sinT], axis=0)
    cosf2 = np.concatenate([cosf, cosf], axis=0)         # [128, T] two heads
    sinf2 = np.concatenate([sinf, sinf], axis=0)

    perm = np.zeros((DR, DR), dtype=np.float32)
    for i in range(DR):
        perm[i, (i + DR // 2) % DR] = 1.0
    perm128 = np.zeros((128, 128), dtype=np.float32)
    perm128[:DR, :DR] = perm
    perm128[DR:, DR:] = perm

    # additive causal mask for the 4 diagonal sub-positions
    maskd = np.zeros((128, 4 * 512), dtype=np.float32)
    p = np.arange(128)[:, None]
    f = np.arange(512)[None, :]
    for sub in range(4):
        maskd[:, sub * 512:(sub + 1) * 512] = np.where(
            p + 128 * sub <= f, 0.0, MASKV)

    # q_b columns per dest: [qn_h0 | qn_h1 | qpe_h0(perm) ; qpe_h1(perm)]
    wqb_all = np.concatenate([
        np.concatenate([
            w_q_b[:, h0 * DQK:h0 * DQK + DN],
            w_q_b[:, h1 * DQK:h1 * DQK + DN],
            w_q_b[:, h0 * DQK + DN:(h0 + 1) * DQK][:, order],
            w_q_b[:, h1 * DQK + DN:(h1 + 1) * DQK][:, order],
        ], axis=1)
        for h0, h1 in ((2 * d, 2 * d + 1) for d in range(NCORES))
    ], axis=1) * q_a_ln_w[:, None] * SCALE

    def pack(w, mrows):
        Kd, Md = w.shape
        n = Md // mrows
        return np.ascontiguousarray(
            w.reshape(Kd // 128, 128, n, mrows).transpose(2, 1, 0, 3)
            .reshape(n * 128, (Kd // 128) * mrows))

    wqa_pk = pack(w_q_a, 128)
    wkva_pk = pack(wkva_p[:, :KVLR], 128)
    wkpe_pk = pack(wkva_p[:, KVLR:], DR)
    wqb_pk = pack(wqb_all, QCH)

    def bf(x):
        return np.ascontiguousarray(np.asarray(x, dtype=np.float32)).astype(BF_NP)

    in_maps = []
    for c in range(NCORES):
        h0, h1 = HPC * c, HPC * c + 1
        # own-head kv_b columns: [kn_h0 | kn_h1 | v_h0 | v_h1], ln folded
        wkvb_own = np.concatenate([
            w_kv_b[:, h0 * (DN + DV):h0 * (DN + DV) + DN],
            w_kv_b[:, h1 * (DN + DV):h1 * (DN + DV) + DN],
            w_kv_b[:, h0 * (DN + DV) + DN:(h0 + 1) * (DN + DV)],
            w_kv_b[:, h1 * (DN + DV) + DN:(h1 + 1) * (DN + DV)],
        ], axis=1) * kv_a_ln_w[:, None]
        wkvb_pk = pack(wkvb_own, 4 * 128)
        wo_c = np.concatenate([
            w_o[h0 * DV:(h0 + 1) * DV, :],
            w_o[h1 * DV:(h1 + 1) * DV, :],
        ], axis=0)
        tsl = slice(c * TSH, (c + 1) * TSH)
        in_maps.append({
            "hs_sh": bf(hs_t[:, tsl]),
            "wqa": bf(wqa_pk),
            "wkva": bf(wkva_pk),
            "wkpe": bf(wkpe_pk),
            "wqb": bf(wqb_pk),
            "wkvb": bf(wkvb_pk),
            "wo": bf(wo_c),
            "cosf2": bf(cosf2[:, tsl]),
            "sinf2": bf(sinf2[:, tsl]),
            "perm128": bf(perm128),
            "ident": bf(np.eye(128, dtype=np.float32)),
            "maskd": bf(maskd),
            "ones": bf(np.ones((128, 128), dtype=np.float32)),
        })
    return in_maps


def kernel(positions, hidden_states, w_q_a, q_a_ln_w, w_q_b, w_kv_a,
           kv_a_ln_w, w_kv_b, w_o):
    nc = _get_nc()
    in_maps = make_in_maps(positions, hidden_states, w_q_a, q_a_ln_w, w_q_b,
                           w_kv_a, kv_a_ln_w, w_kv_b, w_o)
    res = bass_utils.run_bass_kernel_spmd(nc, in_maps, core_ids=list(range(NCORES)))
    acc = np.zeros((T, HID), dtype=np.float32)
    for c in range(NCORES):
        acc += np.asarray(res.results[c]["out"], dtype=np.float32)
    return acc


# revision 18
# speedup vs baseline: 1.9149x; 1.1336x over previous
"""DeepseekV2 MLA attention on 8 Trainium2 NeuronCores (Bass/Tile), v5.

All-bf16 datapath (fp32 PSUM).  Token-sharded front end computes the q/kv
latents on its 256-token shard; the 576-row kv latent (normalized kv_a +
roped k_pe) is AllGathered early so each core expands k_nope/v for its own
2 heads over all 2048 tokens while the q_b outputs (all heads, own shard)
are AllToAll'd.  Attention + row-parallel w_o as in v4; causal mask is
applied additively in PSUM via an identity matmul; host sums partials.
"""

import numpy as np
import ml_dtypes

import concourse.bass as bass
import concourse.bacc as bacc
import concourse.mybir as mybir
import concourse.tile as tile
from concourse import bass_utils

T = 2048
HID = 2048
H = 16
DN = 128
DR = 64
DV = 128
DQK = DN + DR
QLR = 1536
KVLR = 512
THETA = 10000.0
EPS = 1e-6
SCALE = DQK ** -0.5

NCORES = 8
HPC = H // NCORES
LATR = KVLR + DR          # 576 rows of exchanged kv latent

F32 = mybir.dt.float32
BF = mybir.dt.bfloat16
F8 = mybir.dt.float8e4
BF_NP = ml_dtypes.bfloat16
F8_NP = ml_dtypes.float8_e4m3

Q8 = True                 # exchange q_b outputs in fp8e4m3
QE = F8 if Q8 else BF
QE_NP = F8_NP if Q8 else BF_NP

KT = HID // 128           # 16 contraction strips over hidden
QMT = QLR // 128          # 12
KVMT = KVLR // 128        # 4
NB = T // 512             # 4 query blocks
TBT = T // 128            # 16 token blocks
TSH = T // NCORES         # 256 tokens per shard

QCH = 3 * 128             # 384 rows per dest in the q exchange
MASKV = -60.0


def build_bass():
    nc = bacc.Bacc(
        "TRN2",
        target_bir_lowering=False,
        debug=False,
        enable_asserts=False,
        num_devices=NCORES,
    )

    hs_sh = nc.dram_tensor("hs_sh", [HID, TSH], BF, kind="ExternalInput").ap()
    wqa = nc.dram_tensor("wqa", [QMT * 128, KT * 128], BF, kind="ExternalInput").ap()
    wkva = nc.dram_tensor("wkva", [KVMT * 128, KT * 128], BF, kind="ExternalInput").ap()
    wkpe = nc.dram_tensor("wkpe", [128, KT * DR], BF, kind="ExternalInput").ap()
    wqb = nc.dram_tensor("wqb", [NCORES * 128, QMT * QCH], BF, kind="ExternalInput").ap()
    wkvb = nc.dram_tensor("wkvb", [128, KVMT * 4 * 128], BF, kind="ExternalInput").ap()
    wo = nc.dram_tensor("wo", [HPC * DV, HID], BF, kind="ExternalInput").ap()
    cosf2 = nc.dram_tensor("cosf2", [128, TSH], BF, kind="ExternalInput").ap()
    sinf2 = nc.dram_tensor("sinf2", [128, TSH], BF, kind="ExternalInput").ap()
    perm128 = nc.dram_tensor("perm128", [128, 128], BF, kind="ExternalInput").ap()
    ident = nc.dram_tensor("ident", [128, 128], BF, kind="ExternalInput").ap()
    maskd = nc.dram_tensor("maskd", [128, 4 * 512], BF, kind="ExternalInput").ap()
    ones = nc.dram_tensor("ones", [128, 128], BF, kind="ExternalInput").ap()
    out = nc.dram_tensor("out", [T, HID], BF, kind="ExternalOutput").ap()

    with tile.TileContext(nc) as tc:
        _kernel_body(nc, tc, hs_sh, wqa, wkva, wkpe, wqb, wkvb, wo,
                     cosf2, sinf2, perm128, ident, maskd, ones, out)

    nc.compile()
    return nc


def _kernel_body(nc, tc, hs_sh, wqa, wkva, wkpe, wqb, wkvb, wo,
                 cosf2, sinf2, perm128, ident, maskd, ones, out):
    from contextlib import ExitStack

    ctx = ExitStack()
    with ctx:
        dram = ctx.enter_context(tc.tile_pool(name="dram", bufs=1, space="DRAM"))
        contrib_kv = dram.tile([LATR, TSH], BF)
        a2a_kv = dram.tile([NCORES * LATR, TSH], BF)
        contrib_q = dram.tile([NCORES * QCH, TSH], QE)
        a2a_q = dram.tile([NCORES * QCH, TSH], QE)

        persist = ctx.enter_context(tc.tile_pool(name="persist", bufs=1))
        ident_t = persist.tile([128, 128], BF, tag="ident")
        nc.gpsimd.dma_start(out=ident_t, in_=ident)
        perm_t = persist.tile([128, 128], BF, tag="perm")
        nc.gpsimd.dma_start(out=perm_t, in_=perm128)
        cos_t = persist.tile([128, TSH], BF, tag="cos")
        nc.gpsimd.dma_start(out=cos_t, in_=cosf2)
        sin_t = persist.tile([128, TSH], BF, tag="sin")
        nc.gpsimd.dma_start(out=sin_t, in_=sinf2)
        ones_t = persist.tile([128, 128], BF, tag="ones")
        nc.gpsimd.dma_start(out=ones_t, in_=ones)
        maskd_t = persist.tile([128, 4 * 512], BF, tag="maskd")
        nc.gpsimd.dma_start(out=maskd_t, in_=maskd)
        wkvb_t = persist.tile([128, KVMT, 4 * 128], BF, tag="wkvb")
        nc.gpsimd.dma_start(
            out=wkvb_t, in_=wkvb.rearrange("p (s c) -> p s c", s=KVMT))
        wo_t = []
        for h in range(HPC):
            w = persist.tile([128, HID], BF, tag=f"wo{h}")
            nc.gpsimd.dma_start(out=w, in_=wo[h * DV:(h + 1) * DV, :])
            wo_t.append(w)
        ones_col = ones_t[:, 0:1]
        ones_row = ones_t[0:1, :]

        pmid = ctx.enter_context(tc.tile_pool(name="pmid", bufs=1))

        # ---- Phase A: latents on own shard --------------------------------
        with tc.tile_pool(name="pa", bufs=1) as pa, \
             tc.tile_pool(name="psa", bufs=1, space="PSUM") as psa:
            # kv_a weights first so the kv latent (and its AllGather) start
            # as early as possible; hidden strips split across two queues.
            wkva_t = []
            for m in range(KVMT):
                wt = pa.tile([128, KT * 128], BF, tag="wkva", bufs=4,
                             name=f"wkva{m}")
                nc.sync.dma_start(out=wt, in_=wkva[m * 128:(m + 1) * 128, :])
                wkva_t.append(wt)
            wkpe_t = pa.tile([128, KT * DR], BF, tag="wkpe")
            nc.sync.dma_start(out=wkpe_t, in_=wkpe)
            hst = []
            for k in range(KT):
                h = pa.tile([128, TSH], BF, tag=f"hs{k}")
                eng = nc.sync if k % 2 == 0 else nc.gpsimd
                eng.dma_start(out=h, in_=hs_sh[k * 128:(k + 1) * 128, :])
                hst.append(h)

            def rsqrt_bc(z_psum, n, tag):
                tmp = pa.tile([1, TSH], F32, tag="rsq_tmp", bufs=2)
                nc.scalar.activation(tmp, z_psum,
                                     mybir.ActivationFunctionType.Copy,
                                     bias=EPS, scale=1.0 / n)
                nc.vector.reciprocal(tmp, tmp)
                srow = pa.tile([1, TSH], BF, tag=tag + "r", name=tag + "r")
                nc.scalar.activation(srow, tmp,
                                     mybir.ActivationFunctionType.Sqrt)
                b_ps = psa.tile([128, TSH], F32, tag="bc", bufs=1)
                nc.tensor.matmul(b_ps, lhsT=ones_row, rhs=srow,
                                 start=True, stop=True)
                bc = pmid.tile([128, TSH], BF, tag=tag, name=tag)
                nc.scalar.copy(bc, b_ps)
                return bc

            zkv = psa.tile([1, TSH], F32, tag="zkv")
            kv_raw = []   # bf16 un-normalized latent strips
            for m in range(KVMT):
                pq = psa.tile([128, TSH], F32, tag="pq", bufs=3)
                for k in range(KT):
                    nc.tensor.matmul(pq, lhsT=wkva_t[m][:, k * 128:(k + 1) * 128],
                                     rhs=hst[k],
                                     start=(k == 0), stop=(k == KT - 1))
                st = pa.tile([128, TSH], BF, tag=f"kvr{m}", name=f"kvr{m}")
                nc.vector.tensor_copy(st, pq)
                kv_raw.append(st)
                sq = pa.tile([128, TSH], BF, tag="sq", bufs=2)
                nc.vector.tensor_tensor(sq, st, st, op=mybir.AluOpType.mult)
                nc.tensor.matmul(zkv, lhsT=ones_col, rhs=sq,
                                 start=(m == 0), stop=(m == KVMT - 1))
            # raw k_pe
            kpe_ps = psa.tile([DR, TSH], F32, tag="kpeps")
            for k in range(KT):
                nc.tensor.matmul(kpe_ps, lhsT=wkpe_t[:, k * DR:(k + 1) * DR],
                                 rhs=hst[k],
                                 start=(k == 0), stop=(k == KT - 1))
            kpe_raw = pa.tile([DR, TSH], BF, tag="kperaw")
            nc.vector.tensor_copy(kpe_raw, kpe_ps)

            skv_bc = rsqrt_bc(zkv, KVLR, "skvbc")
            # normalized latent staged contiguously for one contrib DMA
            kvstage = pa.tile([128, KVMT, TSH], BF, tag="kvstage")
            for m in range(KVMT):
                nc.vector.tensor_tensor(kvstage[:, m, :], kv_raw[m], skv_bc,
                                        op=mybir.AluOpType.mult)
            # rope k_pe (64 rows; use top half of perm/cos/sin)
            sw_ps = psa.tile([DR, TSH], F32, tag="swk")
            nc.tensor.matmul(sw_ps, lhsT=perm_t[0:DR, 0:DR], rhs=kpe_raw,
                             start=True, stop=True)
            rt1 = pa.tile([DR, TSH], BF, tag="rt1")
            nc.vector.tensor_tensor(rt1, kpe_raw, cos_t[0:DR, :],
                                    op=mybir.AluOpType.mult)
            rt2 = pa.tile([DR, TSH], BF, tag="rt2")
            nc.vector.tensor_tensor(rt2, sw_ps, sin_t[0:DR, :],
                                    op=mybir.AluOpType.mult)
            kpel = pa.tile([DR, TSH], BF, tag="kpel")
            nc.vector.tensor_tensor(kpel, rt1, rt2, op=mybir.AluOpType.add)

            nc.gpsimd.dma_start(
                out=contrib_kv[0:KVLR, :].rearrange("(g p) t -> p g t", p=128),
                in_=kvstage)
            nc.gpsimd.dma_start(out=contrib_kv[KVLR:LATR, :], in_=kpel)
            nc.gpsimd.collective_compute(
                "AllGather", mybir.AluOpType.bypass,
                replica_groups=[list(range(NCORES))],
                ins=[contrib_kv], outs=[a2a_kv])

            # q latent
            zq = psa.tile([1, TSH], F32, tag="zq")
            q_raw = []
            for m in range(QMT):
                wt = pa.tile([128, KT * 128], BF, tag="wqa", bufs=3)
                nc.sync.dma_start(out=wt, in_=wqa[m * 128:(m + 1) * 128, :])
                pq = psa.tile([128, TSH], F32, tag="pq", bufs=3)
                for k in range(KT):
                    nc.tensor.matmul(pq, lhsT=wt[:, k * 128:(k + 1) * 128],
                                     rhs=hst[k],
                                     start=(k == 0), stop=(k == KT - 1))
                st = pmid.tile([128, TSH], BF, tag=f"qr{m}", name=f"qr{m}")
                nc.vector.tensor_copy(st, pq)
                q_raw.append(st)
                sq = pa.tile([128, TSH], BF, tag="sq", bufs=2)
                nc.vector.tensor_tensor(sq, st, st, op=mybir.AluOpType.mult)
                nc.tensor.matmul(zq, lhsT=ones_col, rhs=sq,
                                 start=(m == 0), stop=(m == QMT - 1))
            sq_bc = rsqrt_bc(zq, QLR, "sqbc")
            qan = []
            for m in range(QMT):
                qq = pmid.tile([128, TSH], BF, tag=f"qan{m}", name=f"qan{m}")
                nc.vector.tensor_tensor(qq, q_raw[m], sq_bc,
                                        op=mybir.AluOpType.mult)
                qan.append(qq)

        # ---- q_b for all dests (3 x 128-row tiles per dest) + exchange ----
        with tc.tile_pool(name="pw", bufs=1) as pw, \
             tc.tile_pool(name="psw", bufs=1, space="PSUM") as psw:
            qstage = pw.tile([128, NCORES, 3, TSH], QE, tag="qstage")
            for d in range(NCORES):
                wq = pw.tile([128, QMT * QCH], BF, tag="wq", bufs=2)
                nc.sync.dma_start(out=wq, in_=wqb[d * 128:(d + 1) * 128, :])
                accq = []
                for mt in range(3):
                    a = psw.tile([128, TSH], F32, tag="acc", bufs=6,
                                 name=f"accq{mt}")
                    accq.append(a)
                for k in range(QMT):
                    for mt in range(3):
                        nc.tensor.matmul(
                            accq[mt],
                            lhsT=wq[:, k * QCH + mt * 128:k * QCH + (mt + 1) * 128],
                            rhs=qan[k],
                            start=(k == 0), stop=(k == QMT - 1))
                for hh in range(HPC):
                    nc.vector.tensor_copy(qstage[:, d, hh, :], accq[hh])
                # packed q_pe rope (two heads in one 128-row tile)
                qraw = pw.tile([128, TSH], BF, tag="qraw", bufs=2)
                nc.vector.tensor_copy(qraw, accq[2])
                sw = psw.tile([128, TSH], F32, tag="swq", bufs=2)
                nc.tensor.matmul(sw, lhsT=perm_t, rhs=qraw,
                                 start=True, stop=True)
                r1 = pw.tile([128, TSH], BF, tag="r1", bufs=2)
                nc.vector.tensor_tensor(r1, qraw, cos_t,
                                        op=mybir.AluOpType.mult)
                r2 = pw.tile([128, TSH], BF, tag="r2", bufs=2)
                nc.vector.tensor_tensor(r2, sw, sin_t,
                                        op=mybir.AluOpType.mult)
                nc.vector.tensor_tensor(qstage[:, d, 2, :], r1, r2,
                                        op=mybir.AluOpType.add)
                # stream this dest's chunk out as soon as it is complete
                nc.sync.dma_start(
                    out=contrib_q[d * QCH:(d + 1) * QCH, :].rearrange(
                        "(g p) t -> p g t", p=128),
                    in_=qstage[:, d])
            nc.gpsimd.collective_compute(
                "AllToAll", mybir.AluOpType.bypass,
                replica_groups=[list(range(NCORES))],
                ins=[contrib_q], outs=[a2a_q])

        # ---- Phase B: expand k_nope / v for own heads over all tokens -----
        bcp = ctx.enter_context(tc.tile_pool(name="bcp", bufs=1))
        # NOTE: keep these off the gpsimd queue — instructions behind a
        # collective on the same queue only run after the collective ends.
        kvan = []      # latent strips, all tokens [128, 8, 256]
        for r in range(KVMT):
            kt_ = bcp.tile([128, NCORES, TSH], BF, tag=f"kvan{r}",
                           name=f"kvan{r}")
            nc.sync.dma_start(
                out=kt_,
                in_=a2a_kv.rearrange("(s r) t -> r s t", s=NCORES)
                            [r * 128:(r + 1) * 128])
            kvan.append(kt_)
        kpe_all = bcp.tile([DR, NCORES, TSH], BF, tag="kpe")
        nc.sync.dma_start(
            out=kpe_all,
            in_=a2a_kv.rearrange("(s r) t -> r s t", s=NCORES)[KVLR:LATR])

        def tok512(tile3, c):
            # 512-token chunk c of a [*, 8, 256] tile
            return tile3[:, 2 * c:2 * c + 2, :]

        def tok128(tile3, tb):
            half = (tb % 2) * 128
            return tile3[:, tb // 2, half:half + 128]

        kn = []        # per head [128, 8, 256] feature-major k_nope
        vt = [None] * TBT   # per 128-token block [128, HPC*DV] token-major v
        with tc.tile_pool(name="pb", bufs=1) as pb, \
             tc.tile_pool(name="psb", bufs=1, space="PSUM") as psb:
            for h in range(HPC):
                knh = bcp.tile([128, NCORES, TSH], BF, tag=f"kn{h}",
                               name=f"kn{h}")
                for c in range(4):
                    acck = psb.tile([128, 512], F32, tag="acck", bufs=2)
                    for s in range(KVMT):
                        nc.tensor.matmul(
                            acck, lhsT=wkvb_t[:, s, h * DN:(h + 1) * DN],
                            rhs=tok512(kvan[s], c),
                            start=(s == 0), stop=(s == KVMT - 1))
                    nc.vector.tensor_copy(tok512(knh, c), acck)
                kn.append(knh)
            for tb in range(TBT):
                accv = psb.tile([128, HPC * DV], F32, tag="accv", bufs=3)
                for s in range(KVMT):
                    nc.tensor.matmul(
                        accv, lhsT=tok128(kvan[s], tb),
                        rhs=wkvb_t[:, s, 2 * DN:2 * DN + HPC * DV],
                        start=(s == 0), stop=(s == KVMT - 1))
                vt[tb] = bcp.tile([128, HPC * DV], BF, tag=f"v{tb}",
                                  name=f"v{tb}")
                nc.vector.tensor_copy(vt[tb], accv)

        # q tiles for own heads, all tokens
        qn = []
        for h in range(HPC):
            qh = bcp.tile([128, NCORES, TSH], QE, tag=f"qn{h}", name=f"qn{h}")
            nc.sync.dma_start(
                out=qh,
                in_=a2a_q.rearrange("(s c) t -> c s t", s=NCORES)
                          [h * 128:(h + 1) * 128])
            qn.append(qh)
        qpe = []
        for h in range(HPC):
            qp = bcp.tile([DR, NCORES, TSH], QE, tag=f"qpe{h}", name=f"qpe{h}")
            nc.sync.dma_start(
                out=qp,
                in_=a2a_q.rearrange("(s c) t -> c s t", s=NCORES)
                          [2 * 128 + h * DR:2 * 128 + (h + 1) * DR])
            qpe.append(qp)

        # ---- Attention + output projection --------------------------------
        with tc.tile_pool(name="pc", bufs=1) as pc, \
             tc.tile_pool(name="psc", bufs=1, space="PSUM") as psc:
            attn_n = [[None] * NB for _ in range(HPC)]
            for qj in range(NB):
                nki = 4 * qj + 4
                for h in range(HPC):
                    attn_ps = psc.tile([128, 512], F32, tag="attn", bufs=2)
                    z_ps = psc.tile([1, 512], F32, tag="zr", bufs=1)
                    for ki in range(nki):
                        s_ps = psc.tile([128, 512], F32, tag="s", bufs=2)
                        diag = ki >= 4 * qj
                        nc.tensor.matmul(s_ps, lhsT=tok128(kn[h], ki),
                                         rhs=tok512(qn[h], qj),
                                         start=True, stop=False)
                        nc.tensor.matmul(s_ps, lhsT=tok128(kpe_all, ki),
                                         rhs=tok512(qpe[h], qj),
                                         start=False, stop=not diag)
                        if diag:
                            sub = ki - 4 * qj
                            nc.tensor.matmul(
                                s_ps, lhsT=ident_t,
                                rhs=maskd_t[:, sub * 512:(sub + 1) * 512],
                                start=False, stop=True)
                        e = pc.tile([128, 512], BF, tag="e", bufs=4)
                        nc.scalar.activation(e, s_ps,
                                             mybir.ActivationFunctionType.Exp)
                        nc.tensor.matmul(z_ps, lhsT=ones_col, rhs=e,
                                         start=(ki == 0), stop=(ki == nki - 1))
                        nc.tensor.matmul(attn_ps,
                                         lhsT=vt[ki][:, h * DV:(h + 1) * DV],
                                         rhs=e,
                                         start=(ki == 0), stop=(ki == nki - 1))
                    rz = pc.tile([1, 512], BF, tag="rz", bufs=2)
                    with nc.allow_low_precision(reason="bf16 softmax denom"):
                        nc.vector.reciprocal(rz, z_ps)
                    bc_ps = psc.tile([128, 512], F32, tag="bcs", bufs=1)
                    nc.tensor.matmul(bc_ps, lhsT=ones_row, rhs=rz,
                                     start=True, stop=True)
                    bc_sb = pc.tile([128, 512], BF, tag="bcsb", bufs=2)
                    nc.scalar.copy(bc_sb, bc_ps)
                    attn_n[h][qj] = bcp.tile([128, 512], BF,
                                             tag=f"attn{h}_{qj}",
                                             name=f"attn{h}_{qj}")
                    nc.vector.tensor_tensor(attn_n[h][qj], attn_ps, bc_sb,
                                            op=mybir.AluOpType.mult)

                for tt in range(4):
                    tb = qj * 4 + tt
                    tsl = slice(tt * 128, (tt + 1) * 128)
                    o_row = pc.tile([128, HID], BF, tag="orow", bufs=2)
                    for hb in range(NB):
                        o_ps = psc.tile([128, 512], F32, tag="o", bufs=2)
                        for h in range(HPC):
                            nc.tensor.matmul(
                                o_ps,
                                lhsT=attn_n[h][qj][:, tsl],
                                rhs=wo_t[h][:, hb * 512:(hb + 1) * 512],
                                start=(h == 0),
                                stop=(h == HPC - 1),
                            )
                        nc.vector.tensor_copy(
                            o_row[:, hb * 512:(hb + 1) * 512], o_ps)
                    nc.sync.dma_start(out=out[tb * 128:(tb + 1) * 128, :],
                                      in_=o_row)


_NC_CACHE = {}


def _get_nc():
    if "nc" not in _NC_CACHE:
        _NC_CACHE["nc"] = build_bass()
    return _NC_CACHE["nc"]


def make_in_maps(positions, hidden_states, w_q_a, q_a_ln_w, w_q_b, w_kv_a,
                 kv_a_ln_w, w_kv_b, w_o):
    positions = np.asarray(positions)
    hidden_states = np.asarray(hidden_states, dtype=np.float32)
    w_q_a = np.asarray(w_q_a, dtype=np.float32)
    q_a_ln_w = np.asarray(q_a_ln_w, dtype=np.float32)
    w_q_b = np.asarray(w_q_b, dtype=np.float32)
    w_kv_a = np.asarray(w_kv_a, dtype=np.float32)
    kv_a_ln_w = np.asarray(kv_a_ln_w, dtype=np.float32)
    w_kv_b = np.asarray(w_kv_b, dtype=np.float32)
    w_o = np.asarray(w_o, dtype=np.float32)

    hs_t = np.ascontiguousarray(hidden_states.T)

    order = np.concatenate([np.arange(0, DR, 2), np.arange(1, DR, 2)])

    wkva_p = w_kv_a.copy()
    wkva_p[:, KVLR:] = w_kv_a[:, KVLR:][:, order]

    inv_freq = 1.0 / (THETA ** (np.arange(0, DR, 2, dtype=np.float64) / DR))
    ang = positions.astype(np.float64)[:, None] * inv_freq[None, :]
    cosT = np.cos(ang).T.astype(np.float32)
    sinT = np.sin(ang).T.astype(np.float32)
    cosf = np.concatenate([cosT, cosT], axis=0)          # [64, T]
    sinf = np.concatenate([-sinT, sinT], axis=0)
    cosf2 = np.concatenate([cosf, cosf], axis=0)         # [128, T] two heads
    sinf2 = np.concatenate([sinf, sinf], axis=0)

    perm = np.zeros((DR, DR), dtype=np.float32)
    for i in range(DR):
        perm[i, (i + DR // 2) % DR] = 1.0
    perm128 = np.zeros((128, 128), dtype=np.float32)
    perm128[:DR, :DR] = perm
    perm128[DR:, DR:] = perm

    # additive causal mask for the 4 diagonal sub-positions
    maskd = np.zeros((128, 4 * 512), dtype=np.float32)
    p = np.arange(128)[:, None]
    f = np.arange(512)[None, :]
    for sub in range(4):
        maskd[:, sub * 512:(sub + 1) * 512] = np.where(
            p + 128 * sub <= f, 0.0, MASKV)

    # q_b columns per dest: [qn_h0 | qn_h1 | qpe_h0(perm) ; qpe_h1(perm)]
    wqb_all = np.concatenate([
        np.concatenate([
            w_q_b[:, h0 * DQK:h0 * DQK + DN],
            w_q_b[:, h1 * DQK:h1 * DQK + DN],
            w_q_b[:, h0 * DQK + DN:(h0 + 1) * DQK][:, order],
            w_q_b[:, h1 * DQK + DN:(h1 + 1) * DQK][:, order],
        ], axis=1)
        for h0, h1 in ((2 * d, 2 * d + 1) for d in range(NCORES))
    ], axis=1) * q_a_ln_w[:, None] * SCALE

    def pack(w, mrows):
        Kd, Md = w.shape
        n = Md // mrows
        return np.ascontiguousarray(
            w.reshape(Kd // 128, 128, n, mrows).transpose(2, 1, 0, 3)
            .reshape(n * 128, (Kd // 128) * mrows))

    wqa_pk = pack(w_q_a, 128)
    wkva_pk = pack(wkva_p[:, :KVLR], 128)
    wkpe_pk = pack(wkva_p[:, KVLR:], DR)
    wqb_pk = pack(wqb_all, QCH)

    def bf(x):
        return np.ascontiguousarray(np.asarray(x, dtype=np.float32)).astype(BF_NP)

    in_maps = []
    for c in range(NCORES):
        h0, h1 = HPC * c, HPC * c + 1
        # own-head kv_b columns: [kn_h0 | kn_h1 | v_h0 | v_h1], ln folded
        wkvb_own = np.concatenate([
            w_kv_b[:, h0 * (DN + DV):h0 * (DN + DV) + DN],
            w_kv_b[:, h1 * (DN + DV):h1 * (DN + DV) + DN],
            w_kv_b[:, h0 * (DN + DV) + DN:(h0 + 1) * (DN + DV)],
            w_kv_b[:, h1 * (DN + DV) + DN:(h1 + 1) * (DN + DV)],
        ], axis=1) * kv_a_ln_w[:, None]
        wkvb_pk = pack(wkvb_own, 4 * 128)
        wo_c = np.concatenate([
            w_o[h0 * DV:(h0 + 1) * DV, :],
            w_o[h1 * DV:(h1 + 1) * DV, :],
        ], axis=0)
        tsl = slice(c * TSH, (c + 1) * TSH)
        in_maps.append({
            "hs_sh": bf(hs_t[:, tsl]),
            "wqa": bf(wqa_pk),
            "wkva": bf(wkva_pk),
            "wkpe": bf(wkpe_pk),
            "wqb": bf(wqb_pk),
            "wkvb": bf(wkvb_pk),
            "wo": bf(wo_c),
            "cosf2": bf(cosf2[:, tsl]),
            "sinf2": bf(sinf2[:, tsl]),
            "perm128": bf(perm128),
            "ident": bf(np.eye(128, dtype=np.float32)),
            "maskd": bf(maskd),
            "ones": bf(np.ones((128, 128), dtype=np.float32)),
        })
    return in_maps


def kernel(positions, hidden_states, w_q_a, q_a_ln_w, w_q_b, w_kv_a,
           kv_a_ln_w, w_kv_b, w_o):
    nc = _get_nc()
    in_maps = make_in_maps(positions, hidden_states, w_q_a, q_a_ln_w, w_q_b,
                           w_kv_a, kv_a_ln_w, w_kv_b, w_o)
    res = bass_utils.run_bass_kernel_spmd(nc, in_maps, core_ids=list(range(NCORES)))
    acc = np.zeros((T, HID), dtype=np.float32)
    for c in range(NCORES):
        acc += np.asarray(res.results[c]["out"], dtype=np.float32)
    return acc


# revision 26
# speedup vs baseline: 2.1384x; 1.1167x over previous
"""DeepseekV2 MLA attention on 8 Trainium2 NeuronCores (Bass/Tile), v5.

All-bf16 datapath (fp32 PSUM).  Token-sharded front end computes the q/kv
latents on its 256-token shard; the 576-row kv latent (normalized kv_a +
roped k_pe) is AllGathered early so each core expands k_nope/v for its own
2 heads over all 2048 tokens while the q_b outputs (all heads, own shard)
are AllToAll'd.  Attention + row-parallel w_o as in v4; causal mask is
applied additively in PSUM via an identity matmul; host sums partials.
"""

import numpy as np
import ml_dtypes

import concourse.bass as bass
import concourse.bacc as bacc
import concourse.mybir as mybir
import concourse.tile as tile
from concourse import bass_utils

T = 2048
HID = 2048
H = 16
DN = 128
DR = 64
DV = 128
DQK = DN + DR
QLR = 1536
KVLR = 512
THETA = 10000.0
EPS = 1e-6
SCALE = DQK ** -0.5

NCORES = 8
HPC = H // NCORES
LATR = KVLR + DR          # 576 rows of exchanged kv latent

F32 = mybir.dt.float32
BF = mybir.dt.bfloat16
F8 = mybir.dt.float8e4
BF_NP = ml_dtypes.bfloat16
F8_NP = ml_dtypes.float8_e4m3

Q8 = True                 # exchange q_b outputs in fp8e4m3
QE = F8 if Q8 else BF
QE_NP = F8_NP if Q8 else BF_NP

KT = HID // 128           # 16 contraction strips over hidden
QMT = QLR // 128          # 12
KVMT = KVLR // 128        # 4
NB = T // 512             # 4 query blocks
TBT = T // 128            # 16 token blocks
TSH = T // NCORES         # 256 tokens per shard

QCH = 3 * 128             # 384 rows per dest in the q exchange
MASKV = -60.0


def build_bass():
    nc = bacc.Bacc(
        "TRN2",
        target_bir_lowering=False,
        debug=False,
        enable_asserts=False,
        num_devices=NCORES,
    )

    hs_sh = nc.dram_tensor("hs_sh", [HID, TSH], BF, kind="ExternalInput").ap()
    wqa = nc.dram_tensor("wqa", [QMT * 128, KT * 128], BF, kind="ExternalInput").ap()
    wkva = nc.dram_tensor("wkva", [KVMT * 128, KT * 128], BF, kind="ExternalInput").ap()
    wkpe = nc.dram_tensor("wkpe", [128, KT * DR], BF, kind="ExternalInput").ap()
    wqb = nc.dram_tensor("wqb", [NCORES * 128, QMT * QCH], BF, kind="ExternalInput").ap()
    wkvb = nc.dram_tensor("wkvb", [128, KVMT * 4 * 128], BF, kind="ExternalInput").ap()
    wo = nc.dram_tensor("wo", [HPC * DV, HID], BF, kind="ExternalInput").ap()
    cosf2 = nc.dram_tensor("cosf2", [128, TSH], BF, kind="ExternalInput").ap()
    sinf2 = nc.dram_tensor("sinf2", [128, TSH], BF, kind="ExternalInput").ap()
    perm128 = nc.dram_tensor("perm128", [128, 128], BF, kind="ExternalInput").ap()
    ident = nc.dram_tensor("ident", [128, 128], BF, kind="ExternalInput").ap()
    maskd = nc.dram_tensor("maskd", [128, 4 * 512], BF, kind="ExternalInput").ap()
    ones = nc.dram_tensor("ones", [128, 128], BF, kind="ExternalInput").ap()
    out = nc.dram_tensor("out", [T, HID], BF, kind="ExternalOutput").ap()

    with tile.TileContext(nc) as tc:
        _kernel_body(nc, tc, hs_sh, wqa, wkva, wkpe, wqb, wkvb, wo,
                     cosf2, sinf2, perm128, ident, maskd, ones, out)

    nc.compile()
    return nc


def _kernel_body(nc, tc, hs_sh, wqa, wkva, wkpe, wqb, wkvb, wo,
                 cosf2, sinf2, perm128, ident, maskd, ones, out):
    from contextlib import ExitStack

    ctx = ExitStack()
    with ctx:
        dram = ctx.enter_context(tc.tile_pool(name="dram", bufs=1, space="DRAM"))
        contrib_kv = dram.tile([LATR, TSH], BF)
        a2a_kv = dram.tile([NCORES * LATR, TSH], BF)
        contrib_q = dram.tile([NCORES * QCH, TSH], QE)
        a2a_q = dram.tile([NCORES * QCH, TSH], QE)

        persist = ctx.enter_context(tc.tile_pool(name="persist", bufs=1))
        ident_t = persist.tile([128, 128], BF, tag="ident")
        nc.gpsimd.dma_start(out=ident_t, in_=ident)
        perm_t = persist.tile([128, 128], BF, tag="perm")
        nc.gpsimd.dma_start(out=perm_t, in_=perm128)
        cos_t = persist.tile([128, TSH], BF, tag="cos")
        nc.gpsimd.dma_start(out=cos_t, in_=cosf2)
        sin_t = persist.tile([128, TSH], BF, tag="sin")
        nc.gpsimd.dma_start(out=sin_t, in_=sinf2)
        ones_t = persist.tile([128, 128], BF, tag="ones")
        nc.gpsimd.dma_start(out=ones_t, in_=ones)
        # q_b weights for the first dests ride the otherwise-idle Pool queue
        wq_t = []
        for d in range(NCORES):
            wq_t.append(persist.tile([128, QMT * QCH], BF, tag=f"wq{d}",
                                     name=f"wq{d}"))
        for d in range(4):
            nc.gpsimd.dma_start(out=wq_t[d], in_=wqb[d * 128:(d + 1) * 128, :])
        maskd_t = persist.tile([128, 4 * 512], BF, tag="maskd")
        nc.gpsimd.dma_start(out=maskd_t, in_=maskd)
        wkvb_t = persist.tile([128, KVMT, 4 * 128], BF, tag="wkvb")
        nc.gpsimd.dma_start(
            out=wkvb_t, in_=wkvb.rearrange("p (s c) -> p s c", s=KVMT))
        wo_t = []
        for h in range(HPC):
            w = persist.tile([128, HID], BF, tag=f"wo{h}")
            nc.gpsimd.dma_start(out=w, in_=wo[h * DV:(h + 1) * DV, :])
            wo_t.append(w)
        ones_col = ones_t[:, 0:1]
        ones_row = ones_t[0:1, :]

        pmid = ctx.enter_context(tc.tile_pool(name="pmid", bufs=1))

        # ---- Phase A: latents on own shard --------------------------------
        with tc.tile_pool(name="pa", bufs=1) as pa, \
             tc.tile_pool(name="psa", bufs=1, space="PSUM") as psa:
            # kv_a weights first so the kv latent (and its AllGather) start
            # as early as possible; hidden states come in one fat DMA.
            wkva0_t = pa.tile([128, KT * 128], BF, tag="wkva0")
            nc.sync.dma_start(out=wkva0_t, in_=wkva[0:128, :])
            hs_t = pa.tile([128, KT, TSH], BF, tag="hst")
            nc.sync.dma_start(
                out=hs_t, in_=hs_sh.rearrange("(k p) t -> p k t", p=128))
            hst = [hs_t[:, k, :] for k in range(KT)]
            wkva123_t = pa.tile([128, 3, KT * 128], BF, tag="wkva123")
            nc.sync.dma_start(
                out=wkva123_t,
                in_=wkva[128:, :].rearrange("(s p) c -> p s c", p=128))
            wkva_t = [wkva0_t] + [wkva123_t[:, m, :] for m in range(3)]
            wkpe_t = pa.tile([128, KT * DR], BF, tag="wkpe")
            nc.sync.dma_start(out=wkpe_t, in_=wkpe)

            def rsqrt_bc(z_psum, n, tag):
                tmp = pa.tile([1, TSH], F32, tag="rsq_tmp", bufs=2)
                nc.scalar.activation(tmp, z_psum,
                                     mybir.ActivationFunctionType.Copy,
                                     bias=EPS, scale=1.0 / n)
                nc.vector.reciprocal(tmp, tmp)
                srow = pa.tile([1, TSH], BF, tag=tag + "r", name=tag + "r")
                nc.scalar.activation(srow, tmp,
                                     mybir.ActivationFunctionType.Sqrt)
                b_ps = psa.tile([128, TSH], F32, tag="bc", bufs=1)
                nc.tensor.matmul(b_ps, lhsT=ones_row, rhs=srow,
                                 start=True, stop=True)
                bc = pmid.tile([128, TSH], BF, tag=tag, name=tag)
                nc.scalar.copy(bc, b_ps)
                return bc

            zkv = psa.tile([1, TSH], F32, tag="zkv")
            kv_raw = []   # bf16 un-normalized latent strips
            for m in range(KVMT):
                pq = psa.tile([128, TSH], F32, tag="pq", bufs=3)
                for k in range(KT):
                    nc.tensor.matmul(pq, lhsT=wkva_t[m][:, k * 128:(k + 1) * 128],
                                     rhs=hst[k],
                                     start=(k == 0), stop=(k == KT - 1))
                st = pa.tile([128, TSH], BF, tag=f"kvr{m}", name=f"kvr{m}")
                nc.vector.tensor_copy(st, pq)
                kv_raw.append(st)
                sq = pa.tile([128, TSH], BF, tag="sq", bufs=2)
                nc.vector.tensor_tensor(sq, st, st, op=mybir.AluOpType.mult)
                nc.tensor.matmul(zkv, lhsT=ones_col, rhs=sq,
                                 start=(m == 0), stop=(m == KVMT - 1))
            # raw k_pe
            kpe_ps = psa.tile([DR, TSH], F32, tag="kpeps")
            for k in range(KT):
                nc.tensor.matmul(kpe_ps, lhsT=wkpe_t[:, k * DR:(k + 1) * DR],
                                 rhs=hst[k],
                                 start=(k == 0), stop=(k == KT - 1))
            kpe_raw = pa.tile([DR, TSH], BF, tag="kperaw")
            nc.vector.tensor_copy(kpe_raw, kpe_ps)

            skv_bc = rsqrt_bc(zkv, KVLR, "skvbc")
            # normalized latent staged contiguously for one contrib DMA
            kvstage = pa.tile([128, KVMT, TSH], BF, tag="kvstage")
            for m in range(KVMT):
                nc.vector.tensor_tensor(kvstage[:, m, :], kv_raw[m], skv_bc,
                                        op=mybir.AluOpType.mult)
            # rope k_pe (64 rows; use top half of perm/cos/sin)
            sw_ps = psa.tile([DR, TSH], F32, tag="swk")
            nc.tensor.matmul(sw_ps, lhsT=perm_t[0:DR, 0:DR], rhs=kpe_raw,
                             start=True, stop=True)
            rt1 = pa.tile([DR, TSH], BF, tag="rt1")
            nc.vector.tensor_tensor(rt1, kpe_raw, cos_t[0:DR, :],
                                    op=mybir.AluOpType.mult)
            rt2 = pa.tile([DR, TSH], BF, tag="rt2")
            nc.vector.tensor_tensor(rt2, sw_ps, sin_t[0:DR, :],
                                    op=mybir.AluOpType.mult)
            kpel = pa.tile([DR, TSH], BF, tag="kpel")
            nc.vector.tensor_tensor(kpel, rt1, rt2, op=mybir.AluOpType.add)

            nc.gpsimd.dma_start(
                out=contrib_kv[0:KVLR, :].rearrange("(g p) t -> p g t", p=128),
                in_=kvstage)
            nc.gpsimd.dma_start(out=contrib_kv[KVLR:LATR, :], in_=kpel)
            nc.gpsimd.collective_compute(
                "AllGather", mybir.AluOpType.bypass,
                replica_groups=[list(range(NCORES))],
                ins=[contrib_kv], outs=[a2a_kv])

            # q latent
            zq = psa.tile([1, TSH], F32, tag="zq")
            q_raw = []
            for m in range(QMT):
                wt = pa.tile([128, KT * 128], BF, tag="wqa", bufs=3)
                nc.sync.dma_start(out=wt, in_=wqa[m * 128:(m + 1) * 128, :])
                pq = psa.tile([128, TSH], F32, tag="pq", bufs=3)
                for k in range(KT):
                    nc.tensor.matmul(pq, lhsT=wt[:, k * 128:(k + 1) * 128],
                                     rhs=hst[k],
                                     start=(k == 0), stop=(k == KT - 1))
                st = pmid.tile([128, TSH], BF, tag=f"qr{m}", name=f"qr{m}")
                nc.vector.tensor_copy(st, pq)
                q_raw.append(st)
                sq = pa.tile([128, TSH], BF, tag="sq", bufs=2)
                nc.vector.tensor_tensor(sq, st, st, op=mybir.AluOpType.mult)
                nc.tensor.matmul(zq, lhsT=ones_col, rhs=sq,
                                 start=(m == 0), stop=(m == QMT - 1))
            sq_bc = rsqrt_bc(zq, QLR, "sqbc")
            qan = []
            for m in range(QMT):
                qq = pmid.tile([128, TSH], BF, tag=f"qan{m}", name=f"qan{m}")
                nc.vector.tensor_tensor(qq, q_raw[m], sq_bc,
                                        op=mybir.AluOpType.mult)
                qan.append(qq)

        # ---- q_b for all dests (3 x 128-row tiles per dest) + exchange ----
        with tc.tile_pool(name="pw", bufs=1) as pw, \
             tc.tile_pool(name="psw", bufs=1, space="PSUM") as psw:
            qstage = pw.tile([128, NCORES, 3, TSH], QE, tag="qstage")
            for d in range(4, NCORES):
                nc.sync.dma_start(out=wq_t[d], in_=wqb[d * 128:(d + 1) * 128, :])
            for d in range(NCORES):
                wq = wq_t[d]
                accq = []
                for mt in range(3):
                    a = psw.tile([128, TSH], F32, tag="acc", bufs=6,
                                 name=f"accq{mt}")
                    accq.append(a)
                for k in range(QMT):
                    for mt in range(3):
                        nc.tensor.matmul(
                            accq[mt],
                            lhsT=wq[:, k * QCH + mt * 128:k * QCH + (mt + 1) * 128],
                            rhs=qan[k],
                            start=(k == 0), stop=(k == QMT - 1))
                for hh in range(HPC):
                    nc.vector.tensor_copy(qstage[:, d, hh, :], accq[hh])
                # packed q_pe rope (two heads in one 128-row tile)
                qraw = pw.tile([128, TSH], BF, tag="qraw", bufs=2)
                nc.vector.tensor_copy(qraw, accq[2])
                sw = psw.tile([128, TSH], F32, tag="swq", bufs=2)
                nc.tensor.matmul(sw, lhsT=perm_t, rhs=qraw,
                                 start=True, stop=True)
                r1 = pw.tile([128, TSH], BF, tag="r1", bufs=2)
                nc.vector.tensor_tensor(r1, qraw, cos_t,
                                        op=mybir.AluOpType.mult)
                r2 = pw.tile([128, TSH], BF, tag="r2", bufs=2)
                nc.vector.tensor_tensor(r2, sw, sin_t,
                                        op=mybir.AluOpType.mult)
                nc.vector.tensor_tensor(qstage[:, d, 2, :], r1, r2,
                                        op=mybir.AluOpType.add)
                # stream this dest's chunk out as soon as it is complete
                nc.sync.dma_start(
                    out=contrib_q[d * QCH:(d + 1) * QCH, :].rearrange(
                        "(g p) t -> p g t", p=128),
                    in_=qstage[:, d])
            nc.gpsimd.collective_compute(
                "AllToAll", mybir.AluOpType.bypass,
                replica_groups=[list(range(NCORES))],
                ins=[contrib_q], outs=[a2a_q])

        # ---- Phase B: expand k_nope / v for own heads over all tokens -----
        bcp = ctx.enter_context(tc.tile_pool(name="bcp", bufs=1))
        # NOTE: keep these off the gpsimd queue — instructions behind a
        # collective on the same queue only run after the collective ends.
        kvan = []      # latent strips, all tokens [128, 8, 256]
        for r in range(KVMT):
            kt_ = bcp.tile([128, NCORES, TSH], BF, tag=f"kvan{r}",
                           name=f"kvan{r}")
            eng = nc.sync if r % 2 == 0 else nc.scalar
            eng.dma_start(
                out=kt_,
                in_=a2a_kv.rearrange("(s r) t -> r s t", s=NCORES)
                            [r * 128:(r + 1) * 128])
            kvan.append(kt_)
        kpe_all = bcp.tile([DR, NCORES, TSH], BF, tag="kpe")
        nc.scalar.dma_start(
            out=kpe_all,
            in_=a2a_kv.rearrange("(s r) t -> r s t", s=NCORES)[KVLR:LATR])

        def tok512(tile3, c):
            # 512-token chunk c of a [*, 8, 256] tile
            return tile3[:, 2 * c:2 * c + 2, :]

        def tok128(tile3, tb):
            half = (tb % 2) * 128
            return tile3[:, tb // 2, half:half + 128]

        kn = []        # per head [128, 8, 256] feature-major k_nope
        vt = [None] * TBT   # per 128-token block [128, HPC*DV] token-major v
        with tc.tile_pool(name="pb", bufs=1) as pb, \
             tc.tile_pool(name="psb", bufs=1, space="PSUM") as psb:
            for h in range(HPC):
                knh = bcp.tile([128, NCORES, TSH], BF, tag=f"kn{h}",
                               name=f"kn{h}")
                for c in range(4):
                    acck = psb.tile([128, 512], F32, tag="acck", bufs=2)
                    for s in range(KVMT):
                        nc.tensor.matmul(
                            acck, lhsT=wkvb_t[:, s, h * DN:(h + 1) * DN],
                            rhs=tok512(kvan[s], c),
                            start=(s == 0), stop=(s == KVMT - 1))
                    nc.vector.tensor_copy(tok512(knh, c), acck)
                kn.append(knh)
            for tb in range(TBT):
                accv = psb.tile([128, HPC * DV], F32, tag="accv", bufs=3)
                for s in range(KVMT):
                    nc.tensor.matmul(
                        accv, lhsT=tok128(kvan[s], tb),
                        rhs=wkvb_t[:, s, 2 * DN:2 * DN + HPC * DV],
                        start=(s == 0), stop=(s == KVMT - 1))
                vt[tb] = bcp.tile([128, HPC * DV], BF, tag=f"v{tb}",
                                  name=f"v{tb}")
                nc.vector.tensor_copy(vt[tb], accv)

        # q tiles for own heads, all tokens
        qn = []
        for h in range(HPC):
            qh = bcp.tile([128, NCORES, TSH], QE, tag=f"qn{h}", name=f"qn{h}")
            eng = nc.sync if h == 0 else nc.scalar
            eng.dma_start(
                out=qh,
                in_=a2a_q.rearrange("(s c) t -> c s t", s=NCORES)
                          [h * 128:(h + 1) * 128])
            qn.append(qh)
        qpe = []
        for h in range(HPC):
            qp = bcp.tile([DR, NCORES, TSH], QE, tag=f"qpe{h}", name=f"qpe{h}")
            eng = nc.sync if h == 0 else nc.scalar
            eng.dma_start(
                out=qp,
                in_=a2a_q.rearrange("(s c) t -> c s t", s=NCORES)
                          [2 * 128 + h * DR:2 * 128 + (h + 1) * DR])
            qpe.append(qp)

        # ---- Attention + output projection --------------------------------
        with tc.tile_pool(name="pc", bufs=1) as pc, \
             tc.tile_pool(name="psc", bufs=1, space="PSUM") as psc:
            import concourse.bass_isa as bass_isa
            attn_n = [[None] * NB for _ in range(HPC)]
            for qj in range(NB):
                nki = 4 * qj + 4
                for h in range(HPC):
                    attn_ps = psc.tile([128, 512], F32, tag="attn", bufs=2)
                    z_acc = pc.tile([128, 512], BF, tag="zacc", bufs=2)
                    for ki in range(nki):
                        s_ps = psc.tile([128, 512], F32, tag="s", bufs=3)
                        diag = ki >= 4 * qj
                        nc.tensor.matmul(s_ps, lhsT=tok128(kn[h], ki),
                                         rhs=tok512(qn[h], qj),
                                         start=True, stop=False)
                        nc.tensor.matmul(s_ps, lhsT=tok128(kpe_all, ki),
                                         rhs=tok512(qpe[h], qj),
                                         start=False, stop=not diag)
                        if diag:
                            sub = ki - 4 * qj
                            nc.tensor.matmul(
                                s_ps, lhsT=ident_t,
                                rhs=maskd_t[:, sub * 512:(sub + 1) * 512],
                                start=False, stop=True)
                        e = pc.tile([128, 512], BF, tag="e", bufs=4)
                        nc.scalar.activation(e, s_ps,
                                             mybir.ActivationFunctionType.Exp)
                        # softmax denominator on the (idle) Pool engine
                        zt = pc.tile([128, 512], BF, tag="zt", bufs=3)
                        dst = z_acc if ki == 0 else zt
                        nc.gpsimd.partition_all_reduce(
                            dst, e, channels=128,
                            reduce_op=bass_isa.ReduceOp.add)
                        if ki > 0:
                            nc.vector.tensor_tensor(z_acc, z_acc, zt,
                                                    op=mybir.AluOpType.add)
                        nc.tensor.matmul(attn_ps,
                                         lhsT=vt[ki][:, h * DV:(h + 1) * DV],
                                         rhs=e,
                                         start=(ki == 0), stop=(ki == nki - 1))
                    attn_n[h][qj] = bcp.tile([128, 512], BF,
                                             tag=f"attn{h}_{qj}",
                                             name=f"attn{h}_{qj}")
                    with nc.allow_low_precision(reason="bf16 softmax"):
                        nc.vector.tensor_tensor(attn_n[h][qj], attn_ps, z_acc,
                                                op=mybir.AluOpType.divide)

                for tt in range(4):
                    tb = qj * 4 + tt
                    tsl = slice(tt * 128, (tt + 1) * 128)
                    o_row = pc.tile([128, HID], BF, tag="orow", bufs=2)
                    for hb in range(NB):
                        o_ps = psc.tile([128, 512], F32, tag="o", bufs=2)
                        for h in range(HPC):
                            nc.tensor.matmul(
                                o_ps,
                                lhsT=attn_n[h][qj][:, tsl],
                                rhs=wo_t[h][:, hb * 512:(hb + 1) * 512],
                                start=(h == 0),
                                stop=(h == HPC - 1),
                            )
                        ceng = nc.vector if hb % 2 == 0 else nc.scalar
                        if ceng is nc.vector:
                            ceng.tensor_copy(
                                o_row[:, hb * 512:(hb + 1) * 512], o_ps)
                        else:
                            ceng.copy(o_row[:, hb * 512:(hb + 1) * 512], o_ps)
                    nc.sync.dma_start(out=out[tb * 128:(tb + 1) * 128, :],
                                      in_=o_row)


_NC_CACHE = {}


def _get_nc():
    if "nc" not in _NC_CACHE:
        _NC_CACHE["nc"] = build_bass()
    return _NC_CACHE["nc"]


def make_in_maps(positions, hidden_states, w_q_a, q_a_ln_w, w_q_b, w_kv_a,
                 kv_a_ln_w, w_kv_b, w_o):
    positions = np.asarray(positions)
    hidden_states = np.asarray(hidden_states, dtype=np.float32)
    w_q_a = np.asarray(w_q_a, dtype=np.float32)
    q_a_ln_w = np.asarray(q_a_ln_w, dtype=np.float32)
    w_q_b = np.asarray(w_q_b, dtype=np.float32)
    w_kv_a = np.asarray(w_kv_a, dtype=np.float32)
    kv_a_ln_w = np.asarray(kv_a_ln_w, dtype=np.float32)
    w_kv_b = np.asarray(w_kv_b, dtype=np.float32)
    w_o = np.asarray(w_o, dtype=np.float32)

    hs_t = np.ascontiguousarray(hidden_states.T)

    order = np.concatenate([np.arange(0, DR, 2), np.arange(1, DR, 2)])

    wkva_p = w_kv_a.copy()
    wkva_p[:, KVLR:] = w_kv_a[:, KVLR:][:, order]

    inv_freq = 1.0 / (THETA ** (np.arange(0, DR, 2, dtype=np.float64) / DR))
    ang = positions.astype(np.float64)[:, None] * inv_freq[None, :]
    cosT = np.cos(ang).T.astype(np.float32)
    sinT = np.sin(ang).T.astype(np.float32)
    cosf = np.concatenate([cosT, cosT], axis=0)          # [64, T]
    sinf = np.concatenate([-sinT, sinT], axis=0)
    cosf2 = np.concatenate([cosf, cosf], axis=0)         # [128, T] two heads
    sinf2 = np.concatenate([sinf, sinf], axis=0)

    perm = np.zeros((DR, DR), dtype=np.float32)
    for i in range(DR):
        perm[i, (i + DR // 2) % DR] = 1.0
    perm128 = np.zeros((128, 128), dtype=np.float32)
    perm128[:DR, :DR] = perm
    perm128[DR:, DR:] = perm

    # additive causal mask for the 4 diagonal sub-positions
    maskd = np.zeros((128, 4 * 512), dtype=np.float32)
    p = np.arange(128)[:, None]
    f = np.arange(512)[None, :]
    for sub in range(4):
        maskd[:, sub * 512:(sub + 1) * 512] = np.where(
            p + 128 * sub <= f, 0.0, MASKV)

    # q_b columns per dest: [qn_h0 | qn_h1 | qpe_h0(perm) ; qpe_h1(perm)]
    wqb_all = np.concatenate([
        np.concatenate([
            w_q_b[:, h0 * DQK:h0 * DQK + DN],
            w_q_b[:, h1 * DQK:h1 * DQK + DN],
            w_q_b[:, h0 * DQK + DN:(h0 + 1) * DQK][:, order],
            w_q_b[:, h1 * DQK + DN:(h1 + 1) * DQK][:, order],
        ], axis=1)
        for h0, h1 in ((2 * d, 2 * d + 1) for d in range(NCORES))
    ], axis=1) * q_a_ln_w[:, None] * SCALE

    def pack(w, mrows):
        Kd, Md = w.shape
        n = Md // mrows
        return np.ascontiguousarray(
            w.reshape(Kd // 128, 128, n, mrows).transpose(2, 1, 0, 3)
            .reshape(n * 128, (Kd // 128) * mrows))

    wqa_pk = pack(w_q_a, 128)
    wkva_pk = pack(wkva_p[:, :KVLR], 128)
    wkpe_pk = pack(wkva_p[:, KVLR:], DR)
    wqb_pk = pack(wqb_all, QCH)

    def bf(x):
        return np.ascontiguousarray(np.asarray(x, dtype=np.float32)).astype(BF_NP)

    in_maps = []
    for c in range(NCORES):
        h0, h1 = HPC * c, HPC * c + 1
        # own-head kv_b columns: [kn_h0 | kn_h1 | v_h0 | v_h1], ln folded
        wkvb_own = np.concatenate([
            w_kv_b[:, h0 * (DN + DV):h0 * (DN + DV) + DN],
            w_kv_b[:, h1 * (DN + DV):h1 * (DN + DV) + DN],
            w_kv_b[:, h0 * (DN + DV) + DN:(h0 + 1) * (DN + DV)],
            w_kv_b[:, h1 * (DN + DV) + DN:(h1 + 1) * (DN + DV)],
        ], axis=1) * kv_a_ln_w[:, None]
        wkvb_pk = pack(wkvb_own, 4 * 128)
        wo_c = np.concatenate([
            w_o[h0 * DV:(h0 + 1) * DV, :],
            w_o[h1 * DV:(h1 + 1) * DV, :],
        ], axis=0)
        tsl = slice(c * TSH, (c + 1) * TSH)
        in_maps.append({
            "hs_sh": bf(hs_t[:, tsl]),
            "wqa": bf(wqa_pk),
            "wkva": bf(wkva_pk),
            "wkpe": bf(wkpe_pk),
            "wqb": bf(wqb_pk),
            "wkvb": bf(wkvb_pk),
            "wo": bf(wo_c),
            "cosf2": bf(cosf2[:, tsl]),
            "sinf2": bf(sinf2[:, tsl]),
            "perm128": bf(perm128),
            "ident": bf(np.eye(128, dtype=np.float32)),
            "maskd": bf(maskd),
            "ones": bf(np.ones((128, 128), dtype=np.float32)),
        })
    return in_maps


def kernel(positions, hidden_states, w_q_a, q_a_ln_w, w_q_b, w_kv_a,
           kv_a_ln_w, w_kv_b, w_o):
    nc = _get_nc()
    in_maps = make_in_maps(positions, hidden_states, w_q_a, q_a_ln_w, w_q_b,
                           w_kv_a, kv_a_ln_w, w_kv_b, w_o)
    res = bass_utils.run_bass_kernel_spmd(nc, in_maps, core_ids=list(range(NCORES)))
    acc = np.zeros((T, HID), dtype=np.float32)
    for c in range(NCORES):
        acc += np.asarray(res.results[c]["out"], dtype=np.float32)
    return acc
